# revision 14
# baseline (speedup 1.0000x reference)
"""Trainium2 Bass kernel for a single DeBERTa-style attention head.

Problem shapes (hardcoded):
  B=8, S=2048, E=768(n_embed), H=64(head)
  q = I @ Wq + bq ; k = x @ Wk + bk ; v = x @ Wv + bv
  w = (q @ k^T) / sqrt(E) ; w = where(mask==0, -1e9, w)
  scores = softmax(w, axis=-1) ; out = scores @ v

Sharding: data-parallel over batch B across the 8 NeuronCores.

Design notes (v1 was ~97.6us):
  * HBM stream per core: I/x host-cast to bf16 (6MB), mask uint8 (4MB,
    cast-DMA to bf16).  All bulk DRAM buffers are HOST-PACKED so each
    (partition, chunk) is one contiguous >=3KB run -> SWDGE descriptors
    stay large and the stream runs near the HBM roofline.  fp8 anywhere
    was measured (numpy) at >=2e-2 rel err -> rejected.
  * exp on ACT is the hard floor (~33us for 4.2M logits at 1/lane/cycle);
    the schedule keeps ACT fed from ~13us: 256-col starter chunks, then
    column-progressive I/x interleave, x^T back half before I^T back half
    (k-side projections unlock score pairs 4-7 half-0), masks last (their
    multiplies are exp-gated anyway).
  * q/k projections col-tiled: lhsT=[Wq|Wk] -> [qT;kT] in one PSUM bank,
    two concurrent 64-col PE tiles; for the front half a second swapped
    pass ([Wk|Wq] -> [kT;qT]) builds the duplicate directly on the PE
    (no cross-partition DMA latency on the critical path); the back half
    duplicates via on-chip SBUF->SBUF DMA (latency hidden by then).
  * score matmuls run 2x row-tiled (K=64): ki_a from partitions 0:64 of
    DUP/QK, ki_b from 64:128, emitted interleaved so the PE overlaps them.
  * exp writes straight into the sT tile; mask multiply is an in-place DVE
    tensor_tensor (2x mode) emitted in mask-arrival order.
  * v projection packs 8 seq-chunks per PSUM bank (2 banks), one rank-1
    bias matmul + one DVE copy per bank; denominators fall out of the 65th
    (ones) column of the ctx matmul.
  * ctx accumulation split per q-half; epilogue in two halves so the output
    DMA overlaps the last ctx matmuls; output bf16, host upcasts.
"""

import math
from contextlib import ExitStack

import numpy as np

import concourse.bass as bass
import concourse.tile as tile
import concourse.mybir as mybir
from concourse import bacc
from concourse.bass_utils import run_bass_kernel_spmd

B, S, E, H = 8, 2048, 768, 64
N_CORES = 8
SC = S // 128   # 16 seq chunks
EC = E // 128   # 6 embed chunks
SCALE = 1.0 / math.sqrt(E)

F32 = mybir.dt.float32
BF16 = mybir.dt.bfloat16
U8 = mybir.dt.uint8
AF = mybir.ActivationFunctionType
ALU = mybir.AluOpType

# column chunks for the I/x streams: (lo, len)
CHUNKS = ((0, 256), (256, 256), (512, 512), (1024, 512), (1536, 512))

_cache = {}


def _build_program():
    nc = bacc.Bacc("TRN2", target_bir_lowering=False, debug=False)

    # host-packed: [128, EC*S] where [p, chunk-major (ec, s)] holds
    # I.T[ec*128+p, lo+s] -- one contiguous run per (partition, chunk)
    dIT = nc.dram_tensor("ITp", [128, EC * S], BF16, kind="ExternalInput")
    dXT = nc.dram_tensor("XTp", [128, EC * S], BF16, kind="ExternalInput")
    # host-packed: [128, SC*S] with [p, ki*S + q] = mask.T[ki*128+p, q]
    dmT = nc.dram_tensor("maskTp", [128, SC * S], U8, kind="ExternalInput")
    dWqk = nc.dram_tensor("Wqk", [E, 128], BF16, kind="ExternalInput")
    dWv = nc.dram_tensor("Wv", [E, H], BF16, kind="ExternalInput")
    dbqk = nc.dram_tensor("bqk", [128, 1], F32, kind="ExternalInput")
    dbkq = nc.dram_tensor("bkq", [128, 1], F32, kind="ExternalInput")
    dbv = nc.dram_tensor("bv512", [1, 512], BF16, kind="ExternalInput")
    dout = nc.dram_tensor("out", [S, H], BF16, kind="ExternalOutput")

    chunk_off = {}
    off = 0
    for lo, ln in CHUNKS:
        chunk_off[lo] = off
        off += EC * ln

    with tile.TileContext(nc) as tc, ExitStack() as ctx:
        singles = ctx.enter_context(tc.tile_pool(name="singles", bufs=1))

        IT = singles.tile([128, EC, S], BF16, tag="IT")
        XT = singles.tile([128, EC, S], BF16, tag="XT")

        def load_cols(dst, src, lo, ln):
            o = chunk_off[lo]
            nc.gpsimd.dma_start(
                out=dst[:, :, lo:lo + ln],
                in_=src.ap()[:, o:o + EC * ln].rearrange(
                    "p (ec s) -> p ec s", ec=EC
                ),
            )

        masks = {}
        for mi in range(4):
            masks[mi] = singles.tile(
                [128, 4, S], BF16, name=f"mask{mi}", tag=f"mask{mi}"
            )

        def load_mask(mi):
            nc.gpsimd.dma_start(
                out=masks[mi],
                in_=dmT.ap()[:, mi * 4 * S:(mi + 1) * 4 * S].rearrange(
                    "p (t q) -> p t q", t=4
                ),
            )

        # SWDGE FIFO order == arrival order: column-progressive starters,
        # x^T back half before I^T back half, masks last
        load_cols(IT, dIT, 0, 256)
        load_cols(XT, dXT, 0, 256)
        load_cols(IT, dIT, 256, 256)
        load_cols(XT, dXT, 256, 256)
        load_cols(IT, dIT, 512, 512)
        load_cols(XT, dXT, 512, 512)
        load_mask(0)
        load_cols(XT, dXT, 1024, 512)
        load_cols(XT, dXT, 1536, 512)
        load_cols(IT, dIT, 1024, 512)
        load_cols(IT, dIT, 1536, 512)
        load_mask(1)
        load_mask(2)
        load_mask(3)

        ones_row = singles.tile([1, 512], BF16, tag="ones")
        nc.vector.memset(ones_row, 1.0)

        wqk_sb = singles.tile([128, EC, 128], BF16, tag="Wqk")
        nc.sync.dma_start(
            out=wqk_sb, in_=dWqk.ap().rearrange("(ec p) h -> p ec h", p=128)
        )
        wv_sb = singles.tile([128, EC, H], BF16, tag="Wv")
        nc.sync.dma_start(
            out=wv_sb, in_=dWv.ap().rearrange("(ec p) h -> p ec h", p=128)
        )
        bqk_sb = singles.tile([128, 1], F32, tag="bqk")
        nc.sync.dma_start(out=bqk_sb, in_=dbqk.ap())
        bkq_sb = singles.tile([128, 1], F32, tag="bkq")
        nc.sync.dma_start(out=bkq_sb, in_=dbkq.ap())
        bv_sb = singles.tile([1, 512], BF16, tag="bv512")
        nc.sync.dma_start(out=bv_sb, in_=dbv.ap())

        # QK rows 0:64 = qT, rows 64:128 = kT.  DUP is the partition-swapped
        # copy (rows 0:64 = kT, 64:128 = qT) so both score row-tiles find
        # their operands at the right base partition.
        QK = singles.tile([128, S], BF16, tag="QK")
        DUP = singles.tile([128, S], BF16, tag="DUP")
        vA = singles.tile([128, SC, 66], BF16, tag="vA")
        nc.vector.memset(vA[:, :, H:H + 1], 1.0)

        sp = ctx.enter_context(tc.tile_pool(name="sp", bufs=16))
        psw = ctx.enter_context(tc.tile_pool(name="psw", bufs=2, space="PSUM"))
        outp = ctx.enter_context(tc.tile_pool(name="outp", bufs=1))

        def emit_proj(ps2, lo, ln=512, swap=False):
            """Col-tiled projection for columns lo:lo+ln.

            swap=False: [Wq|Wk] -> [qT;kT] into QK.
            swap=True:  [Wk|Wq] -> [kT;qT] into DUP (PE-side duplicate for
            the front half -- no cross-partition DMA on the critical path).
            """
            dst = DUP if swap else QK
            bias = bkq_sb if swap else bqk_sb
            ps = ps2.tile([128, 512], F32, tag="pqk")
            for ei in range(EC):
                # two col-tile accumulation groups on one bank; each clears
                # its own partition range at ei=0
                w_lo = wqk_sb[:, ei, 64:128] if swap else wqk_sb[:, ei, 0:64]
                w_hi = wqk_sb[:, ei, 0:64] if swap else wqk_sb[:, ei, 64:128]
                r_lo = XT if swap else IT
                r_hi = IT if swap else XT
                nc.tensor.matmul(
                    ps[0:64, 0:ln],
                    lhsT=w_lo,
                    rhs=r_lo[:, ei, lo:lo + ln],
                    start=(ei == 0),
                    stop=(ei == EC - 1),
                    skip_group_check=True,
                )
                nc.tensor.matmul(
                    ps[64:128, 0:ln],
                    lhsT=w_hi,
                    rhs=r_hi[:, ei, lo:lo + ln],
                    start=(ei == 0),
                    stop=(ei == EC - 1),
                    skip_group_check=True,
                )
            nc.vector.tensor_scalar(
                dst[:, lo:lo + ln], ps[:, 0:ln], bias, None, ALU.add
            )

        def emit_halfproj(half, lo):
            """q-only (half=0) or k-only (half=1) projection for the back
            columns; duplicate half via on-chip DMA (latency hidden)."""
            rlo = 64 * half
            ps = psw.tile([128, 1024], F32, tag="w")
            src = IT if half == 0 else XT
            for ei in range(EC):
                nc.tensor.matmul(
                    ps[rlo:rlo + 64, 0:512],
                    lhsT=wqk_sb[:, ei, rlo:rlo + 64],
                    rhs=src[:, ei, lo:lo + 512],
                    start=(ei == 0),
                    stop=(ei == EC - 1),
                )
            nc.vector.tensor_scalar(
                QK[rlo:rlo + 64, lo:lo + 512],
                ps[rlo:rlo + 64, 0:512],
                bqk_sb[rlo:rlo + 64],
                None,
                ALU.add,
            )
            nc.sync.dma_start(
                out=DUP[64 - rlo:128 - rlo, lo:lo + 512],
                in_=QK[rlo:rlo + 64, lo:lo + 512],
            )

        def emit_v_bank(psv, vb):
            """v projection for seq chunks 8*vb..8*vb+7 packed in one bank."""
            ps = psv.tile([128, 512], F32, tag="pv")
            for j in range(8):
                kb = vb * 8 + j
                for ei in range(EC):
                    nc.tensor.matmul(
                        ps[:, j * 64:(j + 1) * 64],
                        lhsT=XT[:, ei, kb * 128:(kb + 1) * 128],
                        rhs=wv_sb[:, ei, :],
                        start=(j == 0 and ei == 0),
                        stop=False,
                    )
            # one rank-1 bias matmul covers all 8 chunks (bv tiled 8x)
            nc.tensor.matmul(
                ps, lhsT=ones_row[:, 0:128], rhs=bv_sb, start=False, stop=True
            )
            nc.vector.tensor_copy(vA[:, vb * 8:(vb + 1) * 8, 0:H], ps)

        sTs = {}
        wtiles = {}

        def emit_wexp(t, hh, split=False):
            """Row-tiled scores + exp for ki pair (2t, 2t+1), q-half hh.

            The two K=64 tiles are emitted interleaved per column sub-chunk
            so the PE runs them concurrently (tile (0,0) from partitions
            0:64 of DUP/QK, tile (64,0) from 64:128).  exp goes straight
            into the sT tiles; mask multiply is emitted separately."""
            ki_a, ki_b = 2 * t, 2 * t + 1
            qlo = hh * 1024
            if ki_a not in sTs:
                sTs[ki_a] = sp.tile([128, S], BF16, name=f"sT{ki_a}", tag="sT")
                sTs[ki_b] = sp.tile([128, S], BF16, name=f"sT{ki_b}", tag="sT")
            subs = split if split else ((0, 512), (512, 512))
            if (t, hh) in wtiles:
                wps = wtiles[t, hh]       # continuation of an earlier call
            else:
                wps = {}
                for ki, rlo in ((ki_a, 0), (ki_b, 64)):
                    wps[ki] = (
                        psw.tile([128, 1024], F32, name=f"w{ki}_{hh}", tag="w"),
                        rlo,
                    )
                wtiles[t, hh] = wps
            for off, ln in subs:
                for ki in (ki_a, ki_b):
                    wp, rlo = wps[ki]
                    src = DUP if rlo == 0 else QK
                    mov = QK if rlo == 0 else DUP
                    nc.tensor.matmul(
                        wp[:, off:off + ln],
                        lhsT=src[rlo:rlo + 64, ki * 128:(ki + 1) * 128],
                        rhs=mov[rlo:rlo + 64, qlo + off:qlo + off + ln],
                        start=True,
                        stop=True,
                    )
                if split:
                    for ki in (ki_a, ki_b):
                        wp, _ = wps[ki]
                        nc.scalar.activation(
                            sTs[ki][:, qlo + off:qlo + off + ln],
                            wp[:, off:off + ln],
                            AF.Exp,
                            scale=SCALE,
                        )
            if not split:
                for ki in (ki_a, ki_b):
                    wp, _ = wps[ki]
                    nc.scalar.activation(
                        sTs[ki][:, qlo:qlo + 1024], wp, AF.Exp, scale=SCALE
                    )

        def emit_mult(t, hh):
            """In-place mask multiply for ki pair (2t, 2t+1), q-half hh."""
            qlo = hh * 1024
            for ki in (2 * t, 2 * t + 1):
                nc.vector.tensor_tensor(
                    sTs[ki][:, qlo:qlo + 1024],
                    sTs[ki][:, qlo:qlo + 1024],
                    masks[ki // 4][:, ki % 4, qlo:qlo + 1024],
                    ALU.mult,
                )

        def emit_ctx(ki, qjs, ctxall):
            sT_sb = sTs[ki]
            for qj in qjs:
                nc.tensor.matmul(
                    ctxall[:, qj, 0:H + 1],
                    lhsT=sT_sb[:, qj * 128:(qj + 1) * 128],
                    rhs=vA[:, ki, 0:H + 1],
                    start=(ki == 0 and qj % 4 == 0),
                    stop=(ki == SC - 1 and qj % 4 == 3),
                )

        with tc.tile_pool(name="ps2", bufs=2, space="PSUM") as ps2, \
             tc.tile_pool(name="psv", bufs=2, space="PSUM") as psv:
            # PE warmup: ~3.5us of rank-1 streams during the initial DMA wait
            # flips the HAM clock-gate to 2.4GHz before real work arrives
            for _ in range(8):
                wt = ps2.tile([128, 512], F32, tag="pqk")
                nc.tensor.matmul(
                    wt, lhsT=ones_row[:, 0:128], rhs=ones_row, start=True, stop=True
                )
            emit_proj(ps2, 0, 256)
            emit_proj(ps2, 0, 256, swap=True)
            emit_proj(ps2, 256, 256)
            emit_proj(ps2, 256, 256, swap=True)
            emit_wexp(0, 0, split=((0, 256), (256, 256)))
            emit_proj(ps2, 512)
            emit_proj(ps2, 512, 512, swap=True)
            emit_wexp(0, 0, split=((512, 512),))
            emit_wexp(1, 0, split=((0, 512), (512, 512)))
            emit_v_bank(psv, 0)
            emit_wexp(2, 0)
            emit_wexp(3, 0)
            emit_halfproj(1, 1024)   # kT back half from x^T (arrives early)
            emit_halfproj(1, 1536)
            emit_v_bank(psv, 1)
            emit_wexp(4, 0)
            emit_wexp(5, 0)
            emit_wexp(6, 0)
            emit_wexp(7, 0)

        # prologue PSUM pools closed -> 4 banks free for ctx accumulation
        psctx = ctx.enter_context(tc.tile_pool(name="psctx", bufs=1, space="PSUM"))
        ctxall = psctx.tile([128, SC, 128], F32, tag="ctxall")
        Q07 = tuple(range(8))
        Q8F = tuple(range(8, SC))

        emit_mult(0, 0)
        emit_mult(1, 0)
        emit_ctx(0, Q07, ctxall)
        emit_ctx(1, Q07, ctxall)
        emit_ctx(2, Q07, ctxall)
        emit_ctx(3, Q07, ctxall)
        emit_halfproj(0, 1024)   # qT back half once I^T lands
        emit_halfproj(0, 1536)
        emit_wexp(0, 1)
        emit_wexp(1, 1)
        # DVE stream in mask/exp-arrival order so it never head-of-line blocks
        emit_mult(0, 1)
        emit_ctx(0, Q8F, ctxall)
        emit_wexp(2, 1)
        emit_mult(2, 0)
        emit_mult(3, 0)
        emit_ctx(4, Q07, ctxall)
        emit_ctx(5, Q07, ctxall)
        emit_ctx(6, Q07, ctxall)
        emit_ctx(7, Q07, ctxall)
        emit_wexp(3, 1)
        emit_mult(1, 1)
        emit_ctx(1, Q8F, ctxall)
        emit_wexp(4, 1)
        emit_mult(4, 0)
        emit_ctx(8, Q07, ctxall)
        emit_ctx(9, Q07, ctxall)
        emit_mult(2, 1)
        emit_ctx(2, Q8F, ctxall)
        emit_wexp(5, 1)
        emit_mult(5, 0)
        emit_ctx(10, Q07, ctxall)
        emit_ctx(11, Q07, ctxall)
        emit_mult(6, 0)
        emit_mult(7, 0)
        emit_ctx(12, Q07, ctxall)
        emit_ctx(13, Q07, ctxall)
        emit_ctx(14, Q07, ctxall)
        emit_ctx(15, Q07, ctxall)
        emit_wexp(6, 1)
        emit_mult(3, 1)
        emit_ctx(3, Q8F, ctxall)
        emit_mult(4, 1)
        emit_ctx(8, Q8F, ctxall)
        emit_ctx(9, Q8F, ctxall)
        emit_wexp(7, 1, split=((0, 512), (512, 512)))
        emit_mult(5, 1)
        emit_ctx(4, Q8F, ctxall)
        emit_ctx(5, Q8F, ctxall)
        emit_ctx(10, Q8F, ctxall)
        emit_ctx(11, Q8F, ctxall)
        emit_mult(6, 1)
        emit_ctx(6, Q8F, ctxall)
        emit_ctx(7, Q8F, ctxall)
        emit_ctx(12, Q8F, ctxall)
        emit_ctx(13, Q8F, ctxall)
        emit_mult(7, 1)
        emit_ctx(14, Q8F, ctxall)

        # epilogue in two halves so output DMA overlaps the last ctx matmuls
        recip_t = outp.tile([128, SC, 1], F32, tag="recip")
        o_all = outp.tile([128, SC, H], BF16, tag="o")

        def emit_epilogue(qlo, qhi):
            nc.vector.reciprocal(
                recip_t[:, qlo:qhi], ctxall[:, qlo:qhi, H:H + 1]
            )
            rb = bass.AP(
                tensor=recip_t.tensor,
                offset=recip_t.offset + qlo * recip_t.ap[1][0],
                ap=[recip_t.ap[0], [recip_t.ap[1][0], qhi - qlo], [0, H]],
            )
            nc.vector.tensor_tensor(
                o_all[:, qlo:qhi], ctxall[:, qlo:qhi, 0:H], rb, ALU.mult
            )
            nc.sync.dma_start(
                out=dout.ap()[qlo * 128:qhi * 128].rearrange(
                    "(qj p) h -> p qj h", p=128
                ),
                in_=o_all[:, qlo:qhi],
            )

        emit_epilogue(0, 8)
        emit_ctx(15, Q8F, ctxall)
        emit_epilogue(8, SC)

    nc.compile()
    return nc


def get_program():
    if "nc" not in _cache:
        _cache["nc"] = _build_program()
    return _cache["nc"]


def _pack_cols(mat_t):
    """[E, S] -> [128, EC*S] chunk-major so each (partition, chunk) run is
    contiguous in DRAM."""
    out = np.empty((128, EC * S), dtype=mat_t.dtype)
    off = 0
    for lo, ln in CHUNKS:
        blk = mat_t[:, lo:lo + ln].reshape(EC, 128, ln).transpose(1, 0, 2)
        out[:, off:off + EC * ln] = blk.reshape(128, EC * ln)
        off += EC * ln
    return out


def make_in_maps(I, x, mask, Wq, bq, Wk, bk, Wv, bv):
    import ml_dtypes

    BF = ml_dtypes.bfloat16
    I = np.asarray(I, dtype=np.float32)
    x = np.asarray(x, dtype=np.float32)
    mask = np.asarray(mask, dtype=np.int32)

    Wqk = np.concatenate(
        [np.asarray(Wq, np.float32), np.asarray(Wk, np.float32)], axis=1
    ).astype(BF)
    Wv_ = np.asarray(Wv, np.float32).astype(BF)
    bq_ = np.asarray(bq, np.float32)
    bk_ = np.asarray(bk, np.float32)
    bqk = np.concatenate([bq_, bk_]).reshape(128, 1).astype(np.float32)
    bkq = np.concatenate([bk_, bq_]).reshape(128, 1).astype(np.float32)
    bv512 = np.tile(np.asarray(bv, np.float32).reshape(1, H), (1, 8)).astype(BF)

    maps = []
    for b in range(B):
        mt = np.ascontiguousarray(mask[b].T).astype(np.uint8)
        maps.append({
            "ITp": _pack_cols(np.ascontiguousarray(I[b].T).astype(BF)),
            "XTp": _pack_cols(np.ascontiguousarray(x[b].T).astype(BF)),
            "maskTp": np.ascontiguousarray(
                mt.reshape(SC, 128, S).transpose(1, 0, 2).reshape(128, SC * S)
            ),
            "Wqk": Wqk, "Wv": Wv_, "bqk": bqk, "bkq": bkq, "bv512": bv512,
        })
    return maps


def kernel(I, x, mask, Wq, bq, Wk, bk, Wv, bv):
    nc = get_program()
    in_maps = make_in_maps(I, x, mask, Wq, bq, Wk, bk, Wv, bv)
    res = run_bass_kernel_spmd(nc, in_maps, list(range(N_CORES)))
    out = np.stack([res.results[b]["out"] for b in range(B)], axis=0)
    return out.astype(np.float32)


# revision 15
# speedup vs baseline: 1.1694x; 1.1694x over previous
"""Trainium2 Bass kernel for a single DeBERTa-style attention head.

Problem shapes (hardcoded):
  B=8, S=2048, E=768(n_embed), H=64(head)
  q = I @ Wq + bq ; k = x @ Wk + bk ; v = x @ Wv + bv
  w = (q @ k^T) / sqrt(E) ; w = where(mask==0, -1e9, w)
  scores = softmax(w, axis=-1) ; out = scores @ v

Sharding: data-parallel over batch B across the 8 NeuronCores.

Design notes (v1 was ~97.6us):
  * HBM stream per core: I/x host-cast to bf16 (6MB), mask uint8 (4MB,
    cast-DMA to bf16).  All bulk DRAM buffers are HOST-PACKED so each
    (partition, chunk) is one contiguous >=3KB run -> SWDGE descriptors
    stay large and the stream runs near the HBM roofline.  fp8 anywhere
    was measured (numpy) at >=2e-2 rel err -> rejected.
  * exp on ACT is the hard floor (~33us for 4.2M logits at 1/lane/cycle);
    the schedule keeps ACT fed from ~13us: 256-col starter chunks, then
    column-progressive I/x interleave, x^T back half before I^T back half
    (k-side projections unlock score pairs 4-7 half-0), masks last (their
    multiplies are exp-gated anyway).
  * q/k projections col-tiled: lhsT=[Wq|Wk] -> [qT;kT] in one PSUM bank,
    two concurrent 64-col PE tiles; for the front half a second swapped
    pass ([Wk|Wq] -> [kT;qT]) builds the duplicate directly on the PE
    (no cross-partition DMA latency on the critical path); the back half
    duplicates via on-chip SBUF->SBUF DMA (latency hidden by then).
  * score matmuls run 2x row-tiled (K=64): ki_a from partitions 0:64 of
    DUP/QK, ki_b from 64:128, emitted interleaved so the PE overlaps them.
  * exp writes straight into the sT tile; mask multiply is an in-place DVE
    tensor_tensor (2x mode) emitted in mask-arrival order.
  * v projection packs 8 seq-chunks per PSUM bank (2 banks), one rank-1
    bias matmul + one DVE copy per bank; denominators fall out of the 65th
    (ones) column of the ctx matmul.
  * ctx accumulation split per q-half; epilogue in two halves so the output
    DMA overlaps the last ctx matmuls; output bf16, host upcasts.
"""

import math
from contextlib import ExitStack

import numpy as np

import concourse.bass as bass
import concourse.tile as tile
import concourse.mybir as mybir
from concourse import bacc
from concourse.bass_utils import run_bass_kernel_spmd

B, S, E, H = 8, 2048, 768, 64
N_CORES = 8
SC = S // 128   # 16 seq chunks
EC = E // 128   # 6 embed chunks
SCALE = 1.0 / math.sqrt(E)

F32 = mybir.dt.float32
BF16 = mybir.dt.bfloat16
U8 = mybir.dt.uint8
AF = mybir.ActivationFunctionType
ALU = mybir.AluOpType

# column chunks for the I/x streams: (lo, len)
CHUNKS = ((0, 256), (256, 256), (512, 512), (1024, 512), (1536, 512))

_cache = {}


def _build_program():
    nc = bacc.Bacc("TRN2", target_bir_lowering=False, debug=False)

    # host-packed: [128, EC*S] where [p, chunk-major (ec, s)] holds
    # I.T[ec*128+p, lo+s] -- one contiguous run per (partition, chunk)
    dIT = nc.dram_tensor("ITp", [128, EC * S], BF16, kind="ExternalInput")
    dXT = nc.dram_tensor("XTp", [128, EC * S], BF16, kind="ExternalInput")
    # host-packed: [128, SC*S] with [p, ki*S + q] = mask.T[ki*128+p, q]
    dmT = nc.dram_tensor("maskTp", [128, SC * S], U8, kind="ExternalInput")
    dWqk = nc.dram_tensor("Wqk", [E, 128], BF16, kind="ExternalInput")
    dWv = nc.dram_tensor("Wv", [E, H], BF16, kind="ExternalInput")
    dbqk = nc.dram_tensor("bqk", [128, 1], F32, kind="ExternalInput")
    dbkq = nc.dram_tensor("bkq", [128, 1], F32, kind="ExternalInput")
    dbv = nc.dram_tensor("bv512", [1, 512], BF16, kind="ExternalInput")
    dout = nc.dram_tensor("out", [S, H], BF16, kind="ExternalOutput")

    chunk_off = {}
    off = 0
    for lo, ln in CHUNKS:
        chunk_off[lo] = off
        off += EC * ln

    with tile.TileContext(nc) as tc, ExitStack() as ctx:
        singles = ctx.enter_context(tc.tile_pool(name="singles", bufs=1))

        # chunk-major [128, EC*S] mirroring the DRAM packing: loads are
        # 1:1 contiguous copies (large descriptors on BOTH sides)
        IT = singles.tile([128, EC * S], BF16, tag="IT")
        XT = singles.tile([128, EC * S], BF16, tag="XT")

        def load_cols(dst, src, lo, ln):
            o = chunk_off[lo]
            nc.gpsimd.dma_start(
                out=dst[:, o:o + EC * ln], in_=src.ap()[:, o:o + EC * ln]
            )

        def itx(tile_, ei, lo, ln):
            """View of packed I/x: [128, ln] covering embed-chunk ei,
            columns lo:lo+ln (must lie within one load chunk)."""
            for clo, cln in CHUNKS:
                if clo <= lo < clo + cln:
                    assert lo + ln <= clo + cln, (lo, ln)
                    o = chunk_off[clo] + ei * cln + (lo - clo)
                    return tile_[:, o:o + ln]
            raise AssertionError(lo)

        masks = {}
        for mi in range(4):
            masks[mi] = singles.tile(
                [128, 4, S], BF16, name=f"mask{mi}", tag=f"mask{mi}"
            )

        def load_mask(mi):
            nc.gpsimd.dma_start(
                out=masks[mi],
                in_=dmT.ap()[:, mi * 4 * S:(mi + 1) * 4 * S].rearrange(
                    "p (t q) -> p t q", t=4
                ),
            )

        # SWDGE FIFO order == arrival order: column-progressive starters,
        # x^T back half before I^T back half, masks last
        load_cols(IT, dIT, 0, 256)
        load_cols(XT, dXT, 0, 256)
        load_cols(IT, dIT, 256, 256)
        load_cols(XT, dXT, 256, 256)
        load_cols(IT, dIT, 512, 512)
        load_cols(XT, dXT, 512, 512)
        load_mask(0)
        load_cols(XT, dXT, 1024, 512)
        load_cols(XT, dXT, 1536, 512)
        load_cols(IT, dIT, 1024, 512)
        load_cols(IT, dIT, 1536, 512)
        load_mask(1)
        load_mask(2)
        load_mask(3)

        ones_row = singles.tile([1, 512], BF16, tag="ones")
        nc.vector.memset(ones_row, 1.0)

        wqk_sb = singles.tile([128, EC, 128], BF16, tag="Wqk")
        nc.sync.dma_start(
            out=wqk_sb, in_=dWqk.ap().rearrange("(ec p) h -> p ec h", p=128)
        )
        wv_sb = singles.tile([128, EC, H], BF16, tag="Wv")
        nc.sync.dma_start(
            out=wv_sb, in_=dWv.ap().rearrange("(ec p) h -> p ec h", p=128)
        )
        bqk_sb = singles.tile([128, 1], F32, tag="bqk")
        nc.sync.dma_start(out=bqk_sb, in_=dbqk.ap())
        bkq_sb = singles.tile([128, 1], F32, tag="bkq")
        nc.sync.dma_start(out=bkq_sb, in_=dbkq.ap())
        bv_sb = singles.tile([1, 512], BF16, tag="bv512")
        nc.sync.dma_start(out=bv_sb, in_=dbv.ap())

        # QK rows 0:64 = qT, rows 64:128 = kT.  DUP is the partition-swapped
        # copy (rows 0:64 = kT, 64:128 = qT) so both score row-tiles find
        # their operands at the right base partition.
        QK = singles.tile([128, S], BF16, tag="QK")
        DUP = singles.tile([128, S], BF16, tag="DUP")
        vA = singles.tile([128, SC, 66], BF16, tag="vA")
        nc.vector.memset(vA[:, :, H:H + 1], 1.0)

        sp = ctx.enter_context(tc.tile_pool(name="sp", bufs=16))
        psw = ctx.enter_context(tc.tile_pool(name="psw", bufs=2, space="PSUM"))
        outp = ctx.enter_context(tc.tile_pool(name="outp", bufs=1))

        def emit_proj(ps2, lo, ln=512, swap=False):
            """Col-tiled projection for columns lo:lo+ln.

            swap=False: [Wq|Wk] -> [qT;kT] into QK.
            swap=True:  [Wk|Wq] -> [kT;qT] into DUP (PE-side duplicate for
            the front half -- no cross-partition DMA on the critical path).
            """
            dst = DUP if swap else QK
            bias = bkq_sb if swap else bqk_sb
            ps = ps2.tile([128, 512], F32, tag="pqk")
            for ei in range(EC):
                # two col-tile accumulation groups on one bank; each clears
                # its own partition range at ei=0
                w_lo = wqk_sb[:, ei, 64:128] if swap else wqk_sb[:, ei, 0:64]
                w_hi = wqk_sb[:, ei, 0:64] if swap else wqk_sb[:, ei, 64:128]
                r_lo = XT if swap else IT
                r_hi = IT if swap else XT
                nc.tensor.matmul(
                    ps[0:64, 0:ln],
                    lhsT=w_lo,
                    rhs=itx(r_lo, ei, lo, ln),
                    start=(ei == 0),
                    stop=(ei == EC - 1),
                    skip_group_check=True,
                )
                nc.tensor.matmul(
                    ps[64:128, 0:ln],
                    lhsT=w_hi,
                    rhs=itx(r_hi, ei, lo, ln),
                    start=(ei == 0),
                    stop=(ei == EC - 1),
                    skip_group_check=True,
                )
            nc.vector.tensor_scalar(
                dst[:, lo:lo + ln], ps[:, 0:ln], bias, None, ALU.add
            )

        def emit_halfproj(half, lo):
            """q-only (half=0) or k-only (half=1) projection for the back
            columns; duplicate half via on-chip DMA (latency hidden)."""
            rlo = 64 * half
            ps = psw.tile([128, 1024], F32, tag="w")
            src = IT if half == 0 else XT
            for ei in range(EC):
                nc.tensor.matmul(
                    ps[rlo:rlo + 64, 0:512],
                    lhsT=wqk_sb[:, ei, rlo:rlo + 64],
                    rhs=itx(src, ei, lo, 512),
                    start=(ei == 0),
                    stop=(ei == EC - 1),
                )
            nc.vector.tensor_scalar(
                QK[rlo:rlo + 64, lo:lo + 512],
                ps[rlo:rlo + 64, 0:512],
                bqk_sb[rlo:rlo + 64],
                None,
                ALU.add,
            )
            nc.sync.dma_start(
                out=DUP[64 - rlo:128 - rlo, lo:lo + 512],
                in_=QK[rlo:rlo + 64, lo:lo + 512],
            )

        def emit_v_bank(psv, vb):
            """v projection for seq chunks 8*vb..8*vb+7 packed in one bank."""
            ps = psv.tile([128, 512], F32, tag="pv")
            for j in range(8):
                kb = vb * 8 + j
                for ei in range(EC):
                    nc.tensor.matmul(
                        ps[:, j * 64:(j + 1) * 64],
                        lhsT=itx(XT, ei, kb * 128, 128),
                        rhs=wv_sb[:, ei, :],
                        start=(j == 0 and ei == 0),
                        stop=False,
                    )
            # one rank-1 bias matmul covers all 8 chunks (bv tiled 8x)
            nc.tensor.matmul(
                ps, lhsT=ones_row[:, 0:128], rhs=bv_sb, start=False, stop=True
            )
            nc.vector.tensor_copy(vA[:, vb * 8:(vb + 1) * 8, 0:H], ps)

        sTs = {}
        wtiles = {}

        def emit_wexp(t, hh, split=False):
            """Row-tiled scores + exp for ki pair (2t, 2t+1), q-half hh.

            The two K=64 tiles are emitted interleaved per column sub-chunk
            so the PE runs them concurrently (tile (0,0) from partitions
            0:64 of DUP/QK, tile (64,0) from 64:128).  exp goes straight
            into the sT tiles; mask multiply is emitted separately."""
            ki_a, ki_b = 2 * t, 2 * t + 1
            qlo = hh * 1024
            if ki_a not in sTs:
                sTs[ki_a] = sp.tile([128, S], BF16, name=f"sT{ki_a}", tag="sT")
                sTs[ki_b] = sp.tile([128, S], BF16, name=f"sT{ki_b}", tag="sT")
            subs = split if split else ((0, 512), (512, 512))
            if (t, hh) in wtiles:
                wps = wtiles[t, hh]       # continuation of an earlier call
            else:
                wps = {}
                for ki, rlo in ((ki_a, 0), (ki_b, 64)):
                    wps[ki] = (
                        psw.tile([128, 1024], F32, name=f"w{ki}_{hh}", tag="w"),
                        rlo,
                    )
                wtiles[t, hh] = wps
            for off, ln in subs:
                for ki in (ki_a, ki_b):
                    wp, rlo = wps[ki]
                    src = DUP if rlo == 0 else QK
                    mov = QK if rlo == 0 else DUP
                    nc.tensor.matmul(
                        wp[:, off:off + ln],
                        lhsT=src[rlo:rlo + 64, ki * 128:(ki + 1) * 128],
                        rhs=mov[rlo:rlo + 64, qlo + off:qlo + off + ln],
                        start=True,
                        stop=True,
                    )
                if split:
                    for ki in (ki_a, ki_b):
                        wp, _ = wps[ki]
                        nc.scalar.activation(
                            sTs[ki][:, qlo + off:qlo + off + ln],
                            wp[:, off:off + ln],
                            AF.Exp,
                            scale=SCALE,
                        )
            if not split:
                for ki in (ki_a, ki_b):
                    wp, _ = wps[ki]
                    nc.scalar.activation(
                        sTs[ki][:, qlo:qlo + 1024], wp, AF.Exp, scale=SCALE
                    )

        def emit_mult(t, hh):
            """In-place mask multiply for ki pair (2t, 2t+1), q-half hh."""
            qlo = hh * 1024
            for ki in (2 * t, 2 * t + 1):
                nc.vector.tensor_tensor(
                    sTs[ki][:, qlo:qlo + 1024],
                    sTs[ki][:, qlo:qlo + 1024],
                    masks[ki // 4][:, ki % 4, qlo:qlo + 1024],
                    ALU.mult,
                )

        def emit_ctx(ki, qjs, ctxall):
            sT_sb = sTs[ki]
            for qj in qjs:
                nc.tensor.matmul(
                    ctxall[:, qj, 0:H + 1],
                    lhsT=sT_sb[:, qj * 128:(qj + 1) * 128],
                    rhs=vA[:, ki, 0:H + 1],
                    start=(ki == 0 and qj % 4 == 0),
                    stop=(ki == SC - 1 and qj % 4 == 3),
                )

        with tc.tile_pool(name="ps2", bufs=2, space="PSUM") as ps2, \
             tc.tile_pool(name="psv", bufs=2, space="PSUM") as psv:
            # PE warmup: ~3.5us of rank-1 streams during the initial DMA wait
            # flips the HAM clock-gate to 2.4GHz before real work arrives
            for _ in range(8):
                wt = ps2.tile([128, 512], F32, tag="pqk")
                nc.tensor.matmul(
                    wt, lhsT=ones_row[:, 0:128], rhs=ones_row, start=True, stop=True
                )
            emit_proj(ps2, 0, 256)
            emit_proj(ps2, 0, 256, swap=True)
            emit_proj(ps2, 256, 256)
            emit_proj(ps2, 256, 256, swap=True)
            emit_wexp(0, 0, split=((0, 256), (256, 256)))
            emit_wexp(1, 0, split=((0, 512),))
            emit_proj(ps2, 512)
            emit_proj(ps2, 512, 512, swap=True)
            emit_wexp(0, 0, split=((512, 512),))
            emit_wexp(1, 0, split=((512, 512),))
            emit_v_bank(psv, 0)
            emit_wexp(2, 0)
            emit_wexp(3, 0)
            emit_halfproj(1, 1024)   # kT back half from x^T (arrives early)
            emit_halfproj(1, 1536)
            emit_v_bank(psv, 1)
            emit_wexp(4, 0)
            emit_wexp(5, 0)
            emit_wexp(6, 0)
            emit_wexp(7, 0)

        # prologue PSUM pools closed -> 4 banks free for ctx accumulation
        psctx = ctx.enter_context(tc.tile_pool(name="psctx", bufs=1, space="PSUM"))
        ctxall = psctx.tile([128, SC, 128], F32, tag="ctxall")
        Q07 = tuple(range(8))
        Q8F = tuple(range(8, SC))

        emit_mult(0, 0)
        emit_mult(1, 0)
        emit_ctx(0, Q07, ctxall)
        emit_ctx(1, Q07, ctxall)
        emit_ctx(2, Q07, ctxall)
        emit_ctx(3, Q07, ctxall)
        emit_halfproj(0, 1024)   # qT back half once I^T lands
        emit_halfproj(0, 1536)
        emit_wexp(0, 1)
        emit_wexp(1, 1)
        # DVE stream in mask/exp-arrival order so it never head-of-line blocks
        emit_mult(0, 1)
        emit_ctx(0, Q8F, ctxall)
        emit_wexp(2, 1)
        emit_mult(2, 0)
        emit_mult(3, 0)
        emit_ctx(4, Q07, ctxall)
        emit_ctx(5, Q07, ctxall)
        emit_ctx(6, Q07, ctxall)
        emit_ctx(7, Q07, ctxall)
        emit_wexp(3, 1)
        emit_mult(1, 1)
        emit_ctx(1, Q8F, ctxall)
        emit_wexp(4, 1)
        emit_mult(4, 0)
        emit_ctx(8, Q07, ctxall)
        emit_ctx(9, Q07, ctxall)
        emit_mult(2, 1)
        emit_ctx(2, Q8F, ctxall)
        emit_wexp(5, 1)
        emit_mult(5, 0)
        emit_ctx(10, Q07, ctxall)
        emit_ctx(11, Q07, ctxall)
        emit_mult(6, 0)
        emit_mult(7, 0)
        emit_ctx(12, Q07, ctxall)
        emit_ctx(13, Q07, ctxall)
        emit_ctx(14, Q07, ctxall)
        emit_ctx(15, Q07, ctxall)
        emit_wexp(6, 1)
        emit_mult(3, 1)
        emit_ctx(3, Q8F, ctxall)
        emit_mult(4, 1)
        emit_ctx(8, Q8F, ctxall)
        emit_ctx(9, Q8F, ctxall)
        emit_wexp(7, 1, split=((0, 512), (512, 512)))
        emit_mult(5, 1)
        emit_ctx(4, Q8F, ctxall)
        emit_ctx(5, Q8F, ctxall)
        emit_ctx(10, Q8F, ctxall)
        emit_ctx(11, Q8F, ctxall)
        emit_mult(6, 1)
        emit_ctx(6, Q8F, ctxall)
        emit_ctx(7, Q8F, ctxall)
        emit_ctx(12, Q8F, ctxall)
        emit_ctx(13, Q8F, ctxall)
        emit_mult(7, 1)
        emit_ctx(14, Q8F, ctxall)

        # epilogue in two halves so output DMA overlaps the last ctx matmuls
        recip_t = outp.tile([128, SC, 1], F32, tag="recip")
        o_all = outp.tile([128, SC, H], BF16, tag="o")

        def emit_epilogue(qlo, qhi):
            nc.vector.reciprocal(
                recip_t[:, qlo:qhi], ctxall[:, qlo:qhi, H:H + 1]
            )
            rb = bass.AP(
                tensor=recip_t.tensor,
                offset=recip_t.offset + qlo * recip_t.ap[1][0],
                ap=[recip_t.ap[0], [recip_t.ap[1][0], qhi - qlo], [0, H]],
            )
            nc.vector.tensor_tensor(
                o_all[:, qlo:qhi], ctxall[:, qlo:qhi, 0:H], rb, ALU.mult
            )
            nc.sync.dma_start(
                out=dout.ap()[qlo * 128:qhi * 128].rearrange(
                    "(qj p) h -> p qj h", p=128
                ),
                in_=o_all[:, qlo:qhi],
            )

        emit_epilogue(0, 8)
        emit_ctx(15, Q8F, ctxall)
        emit_epilogue(8, SC)

    nc.compile()
    return nc


def get_program():
    if "nc" not in _cache:
        _cache["nc"] = _build_program()
    return _cache["nc"]


def _pack_cols(mat_t):
    """[E, S] -> [128, EC*S] chunk-major so each (partition, chunk) run is
    contiguous in DRAM."""
    out = np.empty((128, EC * S), dtype=mat_t.dtype)
    off = 0
    for lo, ln in CHUNKS:
        blk = mat_t[:, lo:lo + ln].reshape(EC, 128, ln).transpose(1, 0, 2)
        out[:, off:off + EC * ln] = blk.reshape(128, EC * ln)
        off += EC * ln
    return out


def make_in_maps(I, x, mask, Wq, bq, Wk, bk, Wv, bv):
    import ml_dtypes

    BF = ml_dtypes.bfloat16
    I = np.asarray(I, dtype=np.float32)
    x = np.asarray(x, dtype=np.float32)
    mask = np.asarray(mask, dtype=np.int32)

    Wqk = np.concatenate(
        [np.asarray(Wq, np.float32), np.asarray(Wk, np.float32)], axis=1
    ).astype(BF)
    Wv_ = np.asarray(Wv, np.float32).astype(BF)
    bq_ = np.asarray(bq, np.float32)
    bk_ = np.asarray(bk, np.float32)
    bqk = np.concatenate([bq_, bk_]).reshape(128, 1).astype(np.float32)
    bkq = np.concatenate([bk_, bq_]).reshape(128, 1).astype(np.float32)
    bv512 = np.tile(np.asarray(bv, np.float32).reshape(1, H), (1, 8)).astype(BF)

    maps = []
    for b in range(B):
        mt = np.ascontiguousarray(mask[b].T).astype(np.uint8)
        maps.append({
            "ITp": _pack_cols(np.ascontiguousarray(I[b].T).astype(BF)),
            "XTp": _pack_cols(np.ascontiguousarray(x[b].T).astype(BF)),
            "maskTp": np.ascontiguousarray(
                mt.reshape(SC, 128, S).transpose(1, 0, 2).reshape(128, SC * S)
            ),
            "Wqk": Wqk, "Wv": Wv_, "bqk": bqk, "bkq": bkq, "bv512": bv512,
        })
    return maps


def kernel(I, x, mask, Wq, bq, Wk, bk, Wv, bv):
    nc = get_program()
    in_maps = make_in_maps(I, x, mask, Wq, bq, Wk, bk, Wv, bv)
    res = run_bass_kernel_spmd(nc, in_maps, list(range(N_CORES)))
    out = np.stack([res.results[b]["out"] for b in range(B)], axis=0)
    return out.astype(np.float32)


# revision 16
# speedup vs baseline: 1.2575x; 1.0753x over previous
"""Trainium2 Bass kernel for a single DeBERTa-style attention head.

Problem shapes (hardcoded):
  B=8, S=2048, E=768(n_embed), H=64(head)
  q = I @ Wq + bq ; k = x @ Wk + bk ; v = x @ Wv + bv
  w = (q @ k^T) / sqrt(E) ; w = where(mask==0, -1e9, w)
  scores = softmax(w, axis=-1) ; out = scores @ v

Sharding: data-parallel over batch B across the 8 NeuronCores.

Design notes (v1 was ~97.6us):
  * HBM stream per core: I/x host-cast to bf16 (6MB), mask uint8 (4MB,
    cast-DMA to bf16).  All bulk DRAM buffers are HOST-PACKED so each
    (partition, chunk) is one contiguous >=3KB run -> SWDGE descriptors
    stay large and the stream runs near the HBM roofline.  fp8 anywhere
    was measured (numpy) at >=2e-2 rel err -> rejected.
  * exp on ACT is the hard floor (~33us for 4.2M logits at 1/lane/cycle);
    the schedule keeps ACT fed from ~13us: 256-col starter chunks, then
    column-progressive I/x interleave, x^T back half before I^T back half
    (k-side projections unlock score pairs 4-7 half-0), masks last (their
    multiplies are exp-gated anyway).
  * q/k projections col-tiled: lhsT=[Wq|Wk] -> [qT;kT] in one PSUM bank,
    two concurrent 64-col PE tiles; for the front half a second swapped
    pass ([Wk|Wq] -> [kT;qT]) builds the duplicate directly on the PE
    (no cross-partition DMA latency on the critical path); the back half
    duplicates via on-chip SBUF->SBUF DMA (latency hidden by then).
  * score matmuls run 2x row-tiled (K=64): ki_a from partitions 0:64 of
    DUP/QK, ki_b from 64:128, emitted interleaved so the PE overlaps them.
  * exp writes straight into the sT tile; mask multiply is an in-place DVE
    tensor_tensor (2x mode) emitted in mask-arrival order.
  * v projection packs 8 seq-chunks per PSUM bank (2 banks), one rank-1
    bias matmul + one DVE copy per bank; denominators fall out of the 65th
    (ones) column of the ctx matmul.
  * ctx accumulation split per q-half; epilogue in two halves so the output
    DMA overlaps the last ctx matmuls; output bf16, host upcasts.
"""

import math
from contextlib import ExitStack

import numpy as np

import concourse.bass as bass
import concourse.tile as tile
import concourse.mybir as mybir
from concourse import bacc
from concourse.bass_utils import run_bass_kernel_spmd

B, S, E, H = 8, 2048, 768, 64
N_CORES = 8
SC = S // 128   # 16 seq chunks
EC = E // 128   # 6 embed chunks
SCALE = 1.0 / math.sqrt(E)

F32 = mybir.dt.float32
BF16 = mybir.dt.bfloat16
U8 = mybir.dt.uint8
AF = mybir.ActivationFunctionType
ALU = mybir.AluOpType

# column chunks for the I/x streams: (lo, len)
CHUNKS = ((0, 256), (256, 256), (512, 512), (1024, 512), (1536, 512))

_cache = {}


def _build_program():
    nc = bacc.Bacc("TRN2", target_bir_lowering=False, debug=False)

    # host-packed: [128, EC*S] where [p, chunk-major (ec, s)] holds
    # I.T[ec*128+p, lo+s] -- one contiguous run per (partition, chunk)
    dIT = nc.dram_tensor("ITp", [128, EC * S], BF16, kind="ExternalInput")
    dXT = nc.dram_tensor("XTp", [128, EC * S], BF16, kind="ExternalInput")
    # host-packed: [128, SC*S] with [p, ki*S + q] = mask.T[ki*128+p, q]
    dmT = nc.dram_tensor("maskTp", [128, SC * S], U8, kind="ExternalInput")
    dWqk = nc.dram_tensor("Wqk", [E, 128], BF16, kind="ExternalInput")
    dWv = nc.dram_tensor("Wv", [E, H], BF16, kind="ExternalInput")
    dbqk = nc.dram_tensor("bqk", [128, 1], F32, kind="ExternalInput")
    dbkq = nc.dram_tensor("bkq", [128, 1], F32, kind="ExternalInput")
    dbv = nc.dram_tensor("bv512", [1, 512], BF16, kind="ExternalInput")
    dout = nc.dram_tensor("out", [S, H], BF16, kind="ExternalOutput")

    chunk_off = {}
    off = 0
    for lo, ln in CHUNKS:
        chunk_off[lo] = off
        off += EC * ln

    with tile.TileContext(nc) as tc, ExitStack() as ctx:
        singles = ctx.enter_context(tc.tile_pool(name="singles", bufs=1))

        # chunk-major [128, EC*S] mirroring the DRAM packing: loads are
        # 1:1 contiguous copies (large descriptors on BOTH sides)
        IT = singles.tile([128, EC * S], BF16, tag="IT")
        XT = singles.tile([128, EC * S], BF16, tag="XT")

        def load_cols(dst, src, lo, ln):
            o = chunk_off[lo]
            nc.gpsimd.dma_start(
                out=dst[:, o:o + EC * ln], in_=src.ap()[:, o:o + EC * ln]
            )

        def itx(tile_, ei, lo, ln):
            """View of packed I/x: [128, ln] covering embed-chunk ei,
            columns lo:lo+ln (must lie within one load chunk)."""
            for clo, cln in CHUNKS:
                if clo <= lo < clo + cln:
                    assert lo + ln <= clo + cln, (lo, ln)
                    o = chunk_off[clo] + ei * cln + (lo - clo)
                    return tile_[:, o:o + ln]
            raise AssertionError(lo)

        masks = {}
        for mi in range(4):
            masks[mi] = singles.tile(
                [128, 4, S], BF16, name=f"mask{mi}", tag=f"mask{mi}"
            )

        def load_mask(mi):
            nc.gpsimd.dma_start(
                out=masks[mi],
                in_=dmT.ap()[:, mi * 4 * S:(mi + 1) * 4 * S].rearrange(
                    "p (t q) -> p t q", t=4
                ),
            )

        # SWDGE FIFO order == arrival order: column-progressive starters,
        # x^T back half before I^T back half, masks last
        load_cols(IT, dIT, 0, 256)
        load_cols(XT, dXT, 0, 256)
        load_cols(IT, dIT, 256, 256)
        load_cols(XT, dXT, 256, 256)
        load_cols(IT, dIT, 512, 512)
        load_cols(XT, dXT, 512, 512)
        load_mask(0)
        load_cols(XT, dXT, 1024, 512)
        load_cols(XT, dXT, 1536, 512)
        load_cols(IT, dIT, 1024, 512)
        load_cols(IT, dIT, 1536, 512)
        load_mask(1)
        load_mask(2)
        load_mask(3)

        ones_row = singles.tile([1, 512], BF16, tag="ones")
        nc.vector.memset(ones_row, 1.0)

        wqk_sb = singles.tile([128, EC, 128], BF16, tag="Wqk")
        nc.sync.dma_start(
            out=wqk_sb, in_=dWqk.ap().rearrange("(ec p) h -> p ec h", p=128)
        )
        wv_sb = singles.tile([128, EC, H], BF16, tag="Wv")
        nc.sync.dma_start(
            out=wv_sb, in_=dWv.ap().rearrange("(ec p) h -> p ec h", p=128)
        )
        bqk_sb = singles.tile([128, 1], F32, tag="bqk")
        nc.sync.dma_start(out=bqk_sb, in_=dbqk.ap())
        bkq_sb = singles.tile([128, 1], F32, tag="bkq")
        nc.sync.dma_start(out=bkq_sb, in_=dbkq.ap())
        bv_sb = singles.tile([1, 512], BF16, tag="bv512")
        nc.sync.dma_start(out=bv_sb, in_=dbv.ap())

        # QK rows 0:64 = qT, rows 64:128 = kT.  DUP is the partition-swapped
        # copy (rows 0:64 = kT, 64:128 = qT) so both score row-tiles find
        # their operands at the right base partition.
        QK = singles.tile([128, S], BF16, tag="QK")
        DUP = singles.tile([128, S], BF16, tag="DUP")
        vA = singles.tile([128, SC, 66], BF16, tag="vA")
        nc.vector.memset(vA[:, :, H:H + 1], 1.0)

        sp = ctx.enter_context(tc.tile_pool(name="sp", bufs=16))
        psw = ctx.enter_context(tc.tile_pool(name="psw", bufs=2, space="PSUM"))
        outp = ctx.enter_context(tc.tile_pool(name="outp", bufs=1))

        def emit_proj(ps2, lo, ln=512, swap=False):
            """Col-tiled projection for columns lo:lo+ln.

            swap=False: [Wq|Wk] -> [qT;kT] into QK.
            swap=True:  [Wk|Wq] -> [kT;qT] into DUP (PE-side duplicate for
            the front half -- no cross-partition DMA on the critical path).
            """
            dst = DUP if swap else QK
            bias = bkq_sb if swap else bqk_sb
            ps = ps2.tile([128, 512], F32, tag="pqk")
            for ei in range(EC):
                # two col-tile accumulation groups on one bank; each clears
                # its own partition range at ei=0
                w_lo = wqk_sb[:, ei, 64:128] if swap else wqk_sb[:, ei, 0:64]
                w_hi = wqk_sb[:, ei, 0:64] if swap else wqk_sb[:, ei, 64:128]
                r_lo = XT if swap else IT
                r_hi = IT if swap else XT
                nc.tensor.matmul(
                    ps[0:64, 0:ln],
                    lhsT=w_lo,
                    rhs=itx(r_lo, ei, lo, ln),
                    start=(ei == 0),
                    stop=(ei == EC - 1),
                    skip_group_check=True,
                )
                nc.tensor.matmul(
                    ps[64:128, 0:ln],
                    lhsT=w_hi,
                    rhs=itx(r_hi, ei, lo, ln),
                    start=(ei == 0),
                    stop=(ei == EC - 1),
                    skip_group_check=True,
                )
            nc.vector.tensor_scalar(
                dst[:, lo:lo + ln], ps[:, 0:ln], bias, None, ALU.add
            )

        def emit_dupproj(half, lo):
            """Back-column projection for one side (half=0: q from I^T,
            half=1: k from x^T), computed TWICE via two col-tiles with the
            same weights so both partition halves (QK and DUP layouts) come
            straight off the PE -- no cross-partition DMA latency."""
            rlo = 64 * half
            ps = psw.tile([128, 1024], F32, tag="w")
            src = IT if half == 0 else XT
            for ei in range(EC):
                w = wqk_sb[:, ei, rlo:rlo + 64]
                r = itx(src, ei, lo, 512)
                nc.tensor.matmul(
                    ps[0:64, 0:512], lhsT=w, rhs=r,
                    start=(ei == 0), stop=(ei == EC - 1),
                    skip_group_check=True,
                )
                nc.tensor.matmul(
                    ps[64:128, 0:512], lhsT=w, rhs=r,
                    start=(ei == 0), stop=(ei == EC - 1),
                    skip_group_check=True,
                )
            bias = bqk_sb[rlo:rlo + 64]
            if half == 0:   # qT -> QK rows 0:64, DUP rows 64:128
                nc.vector.tensor_scalar(
                    QK[0:64, lo:lo + 512], ps[0:64, 0:512], bias, None, ALU.add
                )
                nc.vector.tensor_scalar(
                    DUP[64:128, lo:lo + 512], ps[64:128, 0:512], bias, None, ALU.add
                )
            else:           # kT -> DUP rows 0:64, QK rows 64:128
                nc.vector.tensor_scalar(
                    DUP[0:64, lo:lo + 512], ps[0:64, 0:512], bias, None, ALU.add
                )
                nc.vector.tensor_scalar(
                    QK[64:128, lo:lo + 512], ps[64:128, 0:512], bias, None, ALU.add
                )

        def emit_v_bank(psv, vb):
            """v projection for seq chunks 8*vb..8*vb+7 packed in one bank."""
            ps = psv.tile([128, 512], F32, tag="pv")
            for j in range(8):
                kb = vb * 8 + j
                for ei in range(EC):
                    nc.tensor.matmul(
                        ps[:, j * 64:(j + 1) * 64],
                        lhsT=itx(XT, ei, kb * 128, 128),
                        rhs=wv_sb[:, ei, :],
                        start=(j == 0 and ei == 0),
                        stop=False,
                    )
            # one rank-1 bias matmul covers all 8 chunks (bv tiled 8x)
            nc.tensor.matmul(
                ps, lhsT=ones_row[:, 0:128], rhs=bv_sb, start=False, stop=True
            )
            nc.vector.tensor_copy(vA[:, vb * 8:(vb + 1) * 8, 0:H], ps)

        sTs = {}
        wtiles = {}

        def emit_wexp(t, hh, split=False):
            """Row-tiled scores + exp for ki pair (2t, 2t+1), q-half hh.

            The two K=64 tiles are emitted interleaved per column sub-chunk
            so the PE runs them concurrently (tile (0,0) from partitions
            0:64 of DUP/QK, tile (64,0) from 64:128).  exp goes straight
            into the sT tiles; mask multiply is emitted separately."""
            ki_a, ki_b = 2 * t, 2 * t + 1
            qlo = hh * 1024
            if ki_a not in sTs:
                sTs[ki_a] = sp.tile([128, S], BF16, name=f"sT{ki_a}", tag="sT")
                sTs[ki_b] = sp.tile([128, S], BF16, name=f"sT{ki_b}", tag="sT")
            subs = split if split else ((0, 512), (512, 512))
            if (t, hh) in wtiles:
                wps = wtiles[t, hh]       # continuation of an earlier call
            else:
                wps = {}
                for ki, rlo in ((ki_a, 0), (ki_b, 64)):
                    wps[ki] = (
                        psw.tile([128, 1024], F32, name=f"w{ki}_{hh}", tag="w"),
                        rlo,
                    )
                wtiles[t, hh] = wps
            for off, ln in subs:
                for ki in (ki_a, ki_b):
                    wp, rlo = wps[ki]
                    src = DUP if rlo == 0 else QK
                    mov = QK if rlo == 0 else DUP
                    nc.tensor.matmul(
                        wp[:, off:off + ln],
                        lhsT=src[rlo:rlo + 64, ki * 128:(ki + 1) * 128],
                        rhs=mov[rlo:rlo + 64, qlo + off:qlo + off + ln],
                        start=True,
                        stop=True,
                    )
                if split:
                    for ki in (ki_a, ki_b):
                        wp, _ = wps[ki]
                        nc.scalar.activation(
                            sTs[ki][:, qlo + off:qlo + off + ln],
                            wp[:, off:off + ln],
                            AF.Exp,
                            scale=SCALE,
                        )
            if not split:
                for ki in (ki_a, ki_b):
                    wp, _ = wps[ki]
                    nc.scalar.activation(
                        sTs[ki][:, qlo:qlo + 1024], wp, AF.Exp, scale=SCALE
                    )

        def emit_mult(t, hh):
            """In-place mask multiply for ki pair (2t, 2t+1), q-half hh."""
            qlo = hh * 1024
            for ki in (2 * t, 2 * t + 1):
                nc.vector.tensor_tensor(
                    sTs[ki][:, qlo:qlo + 1024],
                    sTs[ki][:, qlo:qlo + 1024],
                    masks[ki // 4][:, ki % 4, qlo:qlo + 1024],
                    ALU.mult,
                )

        def emit_ctx(ki, qjs, ctxall):
            sT_sb = sTs[ki]
            for qj in qjs:
                nc.tensor.matmul(
                    ctxall[:, qj, 0:H + 1],
                    lhsT=sT_sb[:, qj * 128:(qj + 1) * 128],
                    rhs=vA[:, ki, 0:H + 1],
                    start=(ki == 0 and qj % 4 == 0),
                    stop=(ki == SC - 1 and qj % 4 == 3),
                )

        with tc.tile_pool(name="ps2", bufs=2, space="PSUM") as ps2, \
             tc.tile_pool(name="psv", bufs=2, space="PSUM") as psv:
            # PE warmup: ~3.5us of rank-1 streams during the initial DMA wait
            # flips the HAM clock-gate to 2.4GHz before real work arrives
            for _ in range(8):
                wt = ps2.tile([128, 512], F32, tag="pqk")
                nc.tensor.matmul(
                    wt, lhsT=ones_row[:, 0:128], rhs=ones_row, start=True, stop=True
                )
            emit_proj(ps2, 0, 256)
            emit_proj(ps2, 0, 256, swap=True)
            emit_proj(ps2, 256, 256)
            emit_proj(ps2, 256, 256, swap=True)
            emit_wexp(0, 0, split=((0, 256), (256, 256)))
            emit_wexp(1, 0, split=((0, 512),))
            emit_proj(ps2, 512)
            emit_proj(ps2, 512, 512, swap=True)
            emit_wexp(0, 0, split=((512, 512),))
            emit_wexp(1, 0, split=((512, 512),))
            emit_v_bank(psv, 0)
            emit_wexp(2, 0)
            emit_wexp(3, 0)
            emit_dupproj(1, 1024)   # kT back half from x^T (arrives early)
            emit_dupproj(1, 1536)
            emit_v_bank(psv, 1)
            emit_wexp(4, 0)
            emit_wexp(5, 0)
            emit_wexp(6, 0)
            emit_wexp(7, 0)

        # prologue PSUM pools closed -> 4 banks free for ctx accumulation
        psctx = ctx.enter_context(tc.tile_pool(name="psctx", bufs=1, space="PSUM"))
        ctxall = psctx.tile([128, SC, 128], F32, tag="ctxall")
        Q07 = tuple(range(8))
        Q8F = tuple(range(8, SC))

        emit_mult(0, 0)
        emit_mult(1, 0)
        emit_ctx(0, Q07, ctxall)
        emit_ctx(1, Q07, ctxall)
        emit_ctx(2, Q07, ctxall)
        emit_ctx(3, Q07, ctxall)
        emit_dupproj(0, 1024)   # qT back half once I^T lands
        emit_dupproj(0, 1536)
        emit_wexp(0, 1)
        emit_wexp(1, 1)
        # DVE stream in mask/exp-arrival order so it never head-of-line blocks
        emit_mult(0, 1)
        emit_ctx(0, Q8F, ctxall)
        emit_wexp(2, 1)
        emit_mult(2, 0)
        emit_mult(3, 0)
        emit_ctx(4, Q07, ctxall)
        emit_ctx(5, Q07, ctxall)
        emit_ctx(6, Q07, ctxall)
        emit_ctx(7, Q07, ctxall)
        emit_wexp(3, 1)
        emit_mult(1, 1)
        emit_ctx(1, Q8F, ctxall)
        emit_wexp(4, 1)
        emit_mult(4, 0)
        emit_ctx(8, Q07, ctxall)
        emit_ctx(9, Q07, ctxall)
        emit_mult(2, 1)
        emit_ctx(2, Q8F, ctxall)
        emit_wexp(5, 1)
        emit_mult(5, 0)
        emit_ctx(10, Q07, ctxall)
        emit_ctx(11, Q07, ctxall)
        emit_mult(6, 0)
        emit_mult(7, 0)
        emit_ctx(12, Q07, ctxall)
        emit_ctx(13, Q07, ctxall)
        emit_ctx(14, Q07, ctxall)
        emit_ctx(15, Q07, ctxall)
        emit_wexp(6, 1)
        emit_mult(3, 1)
        emit_ctx(3, Q8F, ctxall)
        emit_mult(4, 1)
        emit_ctx(8, Q8F, ctxall)
        emit_ctx(9, Q8F, ctxall)
        emit_wexp(7, 1, split=((0, 512), (512, 512)))
        emit_mult(5, 1)
        emit_ctx(4, Q8F, ctxall)
        emit_ctx(5, Q8F, ctxall)
        emit_ctx(10, Q8F, ctxall)
        emit_ctx(11, Q8F, ctxall)
        emit_mult(6, 1)
        emit_ctx(6, Q8F, ctxall)
        emit_ctx(7, Q8F, ctxall)
        emit_ctx(12, Q8F, ctxall)
        emit_ctx(13, Q8F, ctxall)
        emit_mult(7, 1)
        emit_ctx(14, Q8F, ctxall)

        # epilogue in two halves so output DMA overlaps the last ctx matmuls
        recip_t = outp.tile([128, SC, 1], F32, tag="recip")
        o_all = outp.tile([128, SC, H], BF16, tag="o")

        def emit_epilogue(qlo, qhi):
            nc.vector.reciprocal(
                recip_t[:, qlo:qhi], ctxall[:, qlo:qhi, H:H + 1]
            )
            rb = bass.AP(
                tensor=recip_t.tensor,
                offset=recip_t.offset + qlo * recip_t.ap[1][0],
                ap=[recip_t.ap[0], [recip_t.ap[1][0], qhi - qlo], [0, H]],
            )
            nc.vector.tensor_tensor(
                o_all[:, qlo:qhi], ctxall[:, qlo:qhi, 0:H], rb, ALU.mult
            )
            nc.sync.dma_start(
                out=dout.ap()[qlo * 128:qhi * 128].rearrange(
                    "(qj p) h -> p qj h", p=128
                ),
                in_=o_all[:, qlo:qhi],
            )

        emit_epilogue(0, 8)
        emit_ctx(15, Q8F, ctxall)
        emit_epilogue(8, SC)

    nc.compile()
    return nc


def get_program():
    if "nc" not in _cache:
        _cache["nc"] = _build_program()
    return _cache["nc"]


def _pack_cols(mat_t):
    """[E, S] -> [128, EC*S] chunk-major so each (partition, chunk) run is
    contiguous in DRAM."""
    out = np.empty((128, EC * S), dtype=mat_t.dtype)
    off = 0
    for lo, ln in CHUNKS:
        blk = mat_t[:, lo:lo + ln].reshape(EC, 128, ln).transpose(1, 0, 2)
        out[:, off:off + EC * ln] = blk.reshape(128, EC * ln)
        off += EC * ln
    return out


def make_in_maps(I, x, mask, Wq, bq, Wk, bk, Wv, bv):
    import ml_dtypes

    BF = ml_dtypes.bfloat16
    I = np.asarray(I, dtype=np.float32)
    x = np.asarray(x, dtype=np.float32)
    mask = np.asarray(mask, dtype=np.int32)

    Wqk = np.concatenate(
        [np.asarray(Wq, np.float32), np.asarray(Wk, np.float32)], axis=1
    ).astype(BF)
    Wv_ = np.asarray(Wv, np.float32).astype(BF)
    bq_ = np.asarray(bq, np.float32)
    bk_ = np.asarray(bk, np.float32)
    bqk = np.concatenate([bq_, bk_]).reshape(128, 1).astype(np.float32)
    bkq = np.concatenate([bk_, bq_]).reshape(128, 1).astype(np.float32)
    bv512 = np.tile(np.asarray(bv, np.float32).reshape(1, H), (1, 8)).astype(BF)

    maps = []
    for b in range(B):
        mt = np.ascontiguousarray(mask[b].T).astype(np.uint8)
        maps.append({
            "ITp": _pack_cols(np.ascontiguousarray(I[b].T).astype(BF)),
            "XTp": _pack_cols(np.ascontiguousarray(x[b].T).astype(BF)),
            "maskTp": np.ascontiguousarray(
                mt.reshape(SC, 128, S).transpose(1, 0, 2).reshape(128, SC * S)
            ),
            "Wqk": Wqk, "Wv": Wv_, "bqk": bqk, "bkq": bkq, "bv512": bv512,
        })
    return maps


def kernel(I, x, mask, Wq, bq, Wk, bk, Wv, bv):
    nc = get_program()
    in_maps = make_in_maps(I, x, mask, Wq, bq, Wk, bk, Wv, bv)
    res = run_bass_kernel_spmd(nc, in_maps, list(range(N_CORES)))
    out = np.stack([res.results[b]["out"] for b in range(B)], axis=0)
    return out.astype(np.float32)


# revision 17
# speedup vs baseline: 1.3125x; 1.0437x over previous
"""Trainium2 Bass kernel for a single DeBERTa-style attention head.

Problem shapes (hardcoded):
  B=8, S=2048, E=768(n_embed), H=64(head)
  q = I @ Wq + bq ; k = x @ Wk + bk ; v = x @ Wv + bv
  w = (q @ k^T) / sqrt(E) ; w = where(mask==0, -1e9, w)
  scores = softmax(w, axis=-1) ; out = scores @ v

Sharding: data-parallel over batch B across the 8 NeuronCores.

Design notes (v1 was ~97.6us):
  * HBM stream per core: I/x host-cast to bf16 (6MB), mask uint8 (4MB,
    cast-DMA to bf16).  All bulk DRAM buffers are HOST-PACKED so each
    (partition, chunk) is one contiguous >=3KB run -> SWDGE descriptors
    stay large and the stream runs near the HBM roofline.  fp8 anywhere
    was measured (numpy) at >=2e-2 rel err -> rejected.
  * exp on ACT is the hard floor (~33us for 4.2M logits at 1/lane/cycle);
    the schedule keeps ACT fed from ~13us: 256-col starter chunks, then
    column-progressive I/x interleave, x^T back half before I^T back half
    (k-side projections unlock score pairs 4-7 half-0), masks last (their
    multiplies are exp-gated anyway).
  * q/k projections col-tiled: lhsT=[Wq|Wk] -> [qT;kT] in one PSUM bank,
    two concurrent 64-col PE tiles; for the front half a second swapped
    pass ([Wk|Wq] -> [kT;qT]) builds the duplicate directly on the PE
    (no cross-partition DMA latency on the critical path); the back half
    duplicates via on-chip SBUF->SBUF DMA (latency hidden by then).
  * score matmuls run 2x row-tiled (K=64): ki_a from partitions 0:64 of
    DUP/QK, ki_b from 64:128, emitted interleaved so the PE overlaps them.
  * exp writes straight into the sT tile; mask multiply is an in-place DVE
    tensor_tensor (2x mode) emitted in mask-arrival order.
  * v projection packs 8 seq-chunks per PSUM bank (2 banks), one rank-1
    bias matmul + one DVE copy per bank; denominators fall out of the 65th
    (ones) column of the ctx matmul.
  * ctx accumulation split per q-half; epilogue in two halves so the output
    DMA overlaps the last ctx matmuls; output bf16, host upcasts.
"""

import math
from contextlib import ExitStack

import numpy as np

import concourse.bass as bass
import concourse.tile as tile
import concourse.mybir as mybir
from concourse import bacc
from concourse.bass_utils import run_bass_kernel_spmd

B, S, E, H = 8, 2048, 768, 64
N_CORES = 8
SC = S // 128   # 16 seq chunks
EC = E // 128   # 6 embed chunks
SCALE = 1.0 / math.sqrt(E)

F32 = mybir.dt.float32
BF16 = mybir.dt.bfloat16
U8 = mybir.dt.uint8
AF = mybir.ActivationFunctionType
ALU = mybir.AluOpType

# column chunks for the I/x streams: (lo, len)
CHUNKS = ((0, 256), (256, 256), (512, 512), (1024, 512), (1536, 512))

_cache = {}


def _build_program():
    nc = bacc.Bacc("TRN2", target_bir_lowering=False, debug=False)

    # host-packed: [128, EC*S] where [p, chunk-major (ec, s)] holds
    # I.T[ec*128+p, lo+s] -- one contiguous run per (partition, chunk)
    dIT = nc.dram_tensor("ITp", [128, EC * S], BF16, kind="ExternalInput")
    dXT = nc.dram_tensor("XTp", [128, EC * S], BF16, kind="ExternalInput")
    # host-packed: [128, SC*S] with [p, ki*S + q] = mask.T[ki*128+p, q]
    dmT = nc.dram_tensor("maskTp", [128, SC * S], U8, kind="ExternalInput")
    dWqk = nc.dram_tensor("Wqk", [E, 128], BF16, kind="ExternalInput")
    dWv = nc.dram_tensor("Wv", [E, H], BF16, kind="ExternalInput")
    dbqk = nc.dram_tensor("bqk", [128, 1], F32, kind="ExternalInput")
    dbkq = nc.dram_tensor("bkq", [128, 1], F32, kind="ExternalInput")
    dbv = nc.dram_tensor("bv512", [1, 512], BF16, kind="ExternalInput")
    dout = nc.dram_tensor("out", [S, H], BF16, kind="ExternalOutput")

    chunk_off = {}
    off = 0
    for lo, ln in CHUNKS:
        chunk_off[lo] = off
        off += EC * ln

    with tile.TileContext(nc) as tc, ExitStack() as ctx:
        singles = ctx.enter_context(tc.tile_pool(name="singles", bufs=1))

        # chunk-major [128, EC*S] mirroring the DRAM packing: loads are
        # 1:1 contiguous copies (large descriptors on BOTH sides)
        IT = singles.tile([128, EC * S], BF16, tag="IT")
        XT = singles.tile([128, EC * S], BF16, tag="XT")

        def load_cols(dst, src, lo, ln):
            o = chunk_off[lo]
            nc.gpsimd.dma_start(
                out=dst[:, o:o + EC * ln], in_=src.ap()[:, o:o + EC * ln]
            )

        def itx(tile_, ei, lo, ln):
            """View of packed I/x: [128, ln] covering embed-chunk ei,
            columns lo:lo+ln (must lie within one load chunk)."""
            for clo, cln in CHUNKS:
                if clo <= lo < clo + cln:
                    assert lo + ln <= clo + cln, (lo, ln)
                    o = chunk_off[clo] + ei * cln + (lo - clo)
                    return tile_[:, o:o + ln]
            raise AssertionError(lo)

        masks = {}
        for mi in range(4):
            masks[mi] = singles.tile(
                [128, 4, S], BF16, name=f"mask{mi}", tag=f"mask{mi}"
            )

        def load_mask(mi):
            nc.gpsimd.dma_start(
                out=masks[mi],
                in_=dmT.ap()[:, mi * 4 * S:(mi + 1) * 4 * S].rearrange(
                    "p (t q) -> p t q", t=4
                ),
            )

        # SWDGE FIFO order == arrival order: column-progressive starters,
        # x^T back half before I^T back half, masks last
        load_cols(IT, dIT, 0, 256)
        load_cols(XT, dXT, 0, 256)
        load_cols(IT, dIT, 256, 256)
        load_cols(XT, dXT, 256, 256)
        load_cols(IT, dIT, 512, 512)
        load_cols(XT, dXT, 512, 512)
        load_mask(0)
        load_cols(XT, dXT, 1024, 512)
        load_cols(XT, dXT, 1536, 512)
        load_cols(IT, dIT, 1024, 512)
        load_cols(IT, dIT, 1536, 512)
        load_mask(1)
        load_mask(2)
        load_mask(3)

        ones_row = singles.tile([1, 512], BF16, tag="ones")
        nc.vector.memset(ones_row, 1.0)

        wqk_sb = singles.tile([128, EC, 128], BF16, tag="Wqk")
        nc.sync.dma_start(
            out=wqk_sb, in_=dWqk.ap().rearrange("(ec p) h -> p ec h", p=128)
        )
        wv_sb = singles.tile([128, EC, H], BF16, tag="Wv")
        nc.sync.dma_start(
            out=wv_sb, in_=dWv.ap().rearrange("(ec p) h -> p ec h", p=128)
        )
        bqk_sb = singles.tile([128, 1], F32, tag="bqk")
        nc.sync.dma_start(out=bqk_sb, in_=dbqk.ap())
        bkq_sb = singles.tile([128, 1], F32, tag="bkq")
        nc.sync.dma_start(out=bkq_sb, in_=dbkq.ap())
        bv_sb = singles.tile([1, 512], BF16, tag="bv512")
        nc.sync.dma_start(out=bv_sb, in_=dbv.ap())

        # QK rows 0:64 = qT, rows 64:128 = kT.  DUP is the partition-swapped
        # copy (rows 0:64 = kT, 64:128 = qT) so both score row-tiles find
        # their operands at the right base partition.
        QK = singles.tile([128, S], BF16, tag="QK")
        DUP = singles.tile([128, S], BF16, tag="DUP")
        vA = singles.tile([128, SC, 66], BF16, tag="vA")
        nc.vector.memset(vA[:, :, H:H + 1], 1.0)

        sp = ctx.enter_context(tc.tile_pool(name="sp", bufs=16))
        # single-bank w tiles, 4 in flight: slot k of score-group g is freed
        # by its exp well before group g+1 reaches the same slot, so the PE
        # w-matmuls hide behind the ACT stream instead of serializing with it
        psw = ctx.enter_context(tc.tile_pool(name="psw", bufs=4, space="PSUM"))
        outp = ctx.enter_context(tc.tile_pool(name="outp", bufs=1))

        def emit_proj(ps2, lo, ln=512, swap=False):
            """Col-tiled projection for columns lo:lo+ln.

            swap=False: [Wq|Wk] -> [qT;kT] into QK.
            swap=True:  [Wk|Wq] -> [kT;qT] into DUP (PE-side duplicate for
            the front half -- no cross-partition DMA on the critical path).
            """
            dst = DUP if swap else QK
            bias = bkq_sb if swap else bqk_sb
            ps = ps2.tile([128, 512], F32, tag="pqk")
            for ei in range(EC):
                # two col-tile accumulation groups on one bank; each clears
                # its own partition range at ei=0
                w_lo = wqk_sb[:, ei, 64:128] if swap else wqk_sb[:, ei, 0:64]
                w_hi = wqk_sb[:, ei, 0:64] if swap else wqk_sb[:, ei, 64:128]
                r_lo = XT if swap else IT
                r_hi = IT if swap else XT
                nc.tensor.matmul(
                    ps[0:64, 0:ln],
                    lhsT=w_lo,
                    rhs=itx(r_lo, ei, lo, ln),
                    start=(ei == 0),
                    stop=(ei == EC - 1),
                    skip_group_check=True,
                )
                nc.tensor.matmul(
                    ps[64:128, 0:ln],
                    lhsT=w_hi,
                    rhs=itx(r_hi, ei, lo, ln),
                    start=(ei == 0),
                    stop=(ei == EC - 1),
                    skip_group_check=True,
                )
            nc.vector.tensor_scalar(
                dst[:, lo:lo + ln], ps[:, 0:ln], bias, None, ALU.add
            )

        def emit_dupproj(half, lo):
            """Back-column projection for one side (half=0: q from I^T,
            half=1: k from x^T), computed TWICE via two col-tiles with the
            same weights so both partition halves (QK and DUP layouts) come
            straight off the PE -- no cross-partition DMA latency."""
            rlo = 64 * half
            ps = psw.tile([128, 512], F32, tag="w")
            src = IT if half == 0 else XT
            for ei in range(EC):
                w = wqk_sb[:, ei, rlo:rlo + 64]
                r = itx(src, ei, lo, 512)
                nc.tensor.matmul(
                    ps[0:64, :], lhsT=w, rhs=r,
                    start=(ei == 0), stop=(ei == EC - 1),
                    skip_group_check=True,
                )
                nc.tensor.matmul(
                    ps[64:128, :], lhsT=w, rhs=r,
                    start=(ei == 0), stop=(ei == EC - 1),
                    skip_group_check=True,
                )
            bias = bqk_sb[rlo:rlo + 64]
            if half == 0:   # qT -> QK rows 0:64, DUP rows 64:128
                nc.vector.tensor_scalar(
                    QK[0:64, lo:lo + 512], ps[0:64, :], bias, None, ALU.add
                )
                nc.vector.tensor_scalar(
                    DUP[64:128, lo:lo + 512], ps[64:128, :], bias, None, ALU.add
                )
            else:           # kT -> DUP rows 0:64, QK rows 64:128
                nc.vector.tensor_scalar(
                    DUP[0:64, lo:lo + 512], ps[0:64, :], bias, None, ALU.add
                )
                nc.vector.tensor_scalar(
                    QK[64:128, lo:lo + 512], ps[64:128, :], bias, None, ALU.add
                )

        def emit_v_bank(psv, vb):
            """v projection for seq chunks 8*vb..8*vb+7 packed in one bank."""
            ps = psv.tile([128, 512], F32, tag="pv")
            for j in range(8):
                kb = vb * 8 + j
                for ei in range(EC):
                    nc.tensor.matmul(
                        ps[:, j * 64:(j + 1) * 64],
                        lhsT=itx(XT, ei, kb * 128, 128),
                        rhs=wv_sb[:, ei, :],
                        start=(j == 0 and ei == 0),
                        stop=False,
                    )
            # one rank-1 bias matmul covers all 8 chunks (bv tiled 8x)
            nc.tensor.matmul(
                ps, lhsT=ones_row[:, 0:128], rhs=bv_sb, start=False, stop=True
            )
            nc.vector.tensor_copy(vA[:, vb * 8:(vb + 1) * 8, 0:H], ps)

        sTs = {}

        def emit_wexp(t, hh, split=False):
            """Row-tiled scores + exp for ki pair (2t, 2t+1), q-half hh.

            Each column sub-chunk gets its own single-bank PSUM tile per ki;
            the two K=64 row-tiles (ki_a at partitions 0:64 of DUP/QK, ki_b
            at 64:128) are emitted back-to-back so the PE overlaps them.
            exp writes straight into the sT tiles; the mask multiply is
            emitted separately in mask-arrival order."""
            ki_a, ki_b = 2 * t, 2 * t + 1
            qlo = hh * 1024
            if ki_a not in sTs:
                sTs[ki_a] = sp.tile([128, S], BF16, name=f"sT{ki_a}", tag="sT")
                sTs[ki_b] = sp.tile([128, S], BF16, name=f"sT{ki_b}", tag="sT")
            subs = split if split else ((0, 512), (512, 512))
            for off, ln in subs:
                wps = {}
                for ki, rlo in ((ki_a, 0), (ki_b, 64)):
                    wps[ki] = psw.tile(
                        [128, 512], F32, name=f"w{ki}_{hh}_{off}", tag="w"
                    )
                    src = DUP if rlo == 0 else QK
                    mov = QK if rlo == 0 else DUP
                    nc.tensor.matmul(
                        wps[ki][:, 0:ln],
                        lhsT=src[rlo:rlo + 64, ki * 128:(ki + 1) * 128],
                        rhs=mov[rlo:rlo + 64, qlo + off:qlo + off + ln],
                        start=True,
                        stop=True,
                    )
                for ki in (ki_a, ki_b):
                    nc.scalar.activation(
                        sTs[ki][:, qlo + off:qlo + off + ln],
                        wps[ki][:, 0:ln],
                        AF.Exp,
                        scale=SCALE,
                    )

        def emit_mult(t, hh):
            """In-place mask multiply for ki pair (2t, 2t+1), q-half hh."""
            qlo = hh * 1024
            for ki in (2 * t, 2 * t + 1):
                nc.vector.tensor_tensor(
                    sTs[ki][:, qlo:qlo + 1024],
                    sTs[ki][:, qlo:qlo + 1024],
                    masks[ki // 4][:, ki % 4, qlo:qlo + 1024],
                    ALU.mult,
                )

        def emit_ctx(ki, qjs, ctxall):
            sT_sb = sTs[ki]
            for qj in qjs:
                nc.tensor.matmul(
                    ctxall[:, qj, 0:H + 1],
                    lhsT=sT_sb[:, qj * 128:(qj + 1) * 128],
                    rhs=vA[:, ki, 0:H + 1],
                    start=(ki == 0 and qj % 4 == 0),
                    stop=(ki == SC - 1 and qj % 4 == 3),
                )

        with tc.tile_pool(name="ps2", bufs=2, space="PSUM") as ps2, \
             tc.tile_pool(name="psv", bufs=1, space="PSUM") as psv:
            # PE warmup: ~3.5us of rank-1 streams during the initial DMA wait
            # flips the HAM clock-gate to 2.4GHz before real work arrives
            for _ in range(8):
                wt = ps2.tile([128, 512], F32, tag="pqk")
                nc.tensor.matmul(
                    wt, lhsT=ones_row[:, 0:128], rhs=ones_row, start=True, stop=True
                )
            emit_proj(ps2, 0, 256)
            emit_proj(ps2, 0, 256, swap=True)
            emit_proj(ps2, 256, 256)
            emit_proj(ps2, 256, 256, swap=True)
            emit_wexp(0, 0, split=((0, 256), (256, 256)))
            emit_wexp(1, 0, split=((0, 512),))
            emit_proj(ps2, 512)
            emit_proj(ps2, 512, 512, swap=True)
            emit_wexp(0, 0, split=((512, 512),))
            emit_wexp(1, 0, split=((512, 512),))
            emit_wexp(2, 0, split=((0, 512), (512, 512)))
            emit_wexp(3, 0, split=((0, 512), (512, 512)))
            emit_v_bank(psv, 0)
            emit_dupproj(1, 1024)   # kT back half from x^T (arrives early)
            emit_dupproj(1, 1536)
            emit_v_bank(psv, 1)
            emit_wexp(4, 0)
            emit_wexp(5, 0)
            emit_wexp(6, 0)
            emit_wexp(7, 0)

        # prologue PSUM pools closed -> 4 banks free for ctx accumulation
        psctx = ctx.enter_context(tc.tile_pool(name="psctx", bufs=1, space="PSUM"))
        ctxall = psctx.tile([128, SC, 128], F32, tag="ctxall")
        Q07 = tuple(range(8))
        Q8F = tuple(range(8, SC))

        emit_mult(0, 0)
        emit_mult(1, 0)
        emit_ctx(0, Q07, ctxall)
        emit_ctx(1, Q07, ctxall)
        emit_ctx(2, Q07, ctxall)
        emit_ctx(3, Q07, ctxall)
        emit_dupproj(0, 1024)   # qT back half once I^T lands
        emit_dupproj(0, 1536)
        emit_wexp(0, 1)
        emit_wexp(1, 1)
        # DVE stream in mask/exp-arrival order so it never head-of-line blocks
        emit_mult(0, 1)
        emit_ctx(0, Q8F, ctxall)
        emit_wexp(2, 1)
        emit_mult(2, 0)
        emit_mult(3, 0)
        emit_ctx(4, Q07, ctxall)
        emit_ctx(5, Q07, ctxall)
        emit_ctx(6, Q07, ctxall)
        emit_ctx(7, Q07, ctxall)
        emit_wexp(3, 1)
        emit_mult(1, 1)
        emit_ctx(1, Q8F, ctxall)
        emit_wexp(4, 1)
        emit_mult(4, 0)
        emit_ctx(8, Q07, ctxall)
        emit_ctx(9, Q07, ctxall)
        emit_mult(2, 1)
        emit_ctx(2, Q8F, ctxall)
        emit_wexp(5, 1)
        emit_mult(5, 0)
        emit_ctx(10, Q07, ctxall)
        emit_ctx(11, Q07, ctxall)
        emit_mult(6, 0)
        emit_mult(7, 0)
        emit_ctx(12, Q07, ctxall)
        emit_ctx(13, Q07, ctxall)
        emit_ctx(14, Q07, ctxall)
        emit_ctx(15, Q07, ctxall)
        emit_wexp(6, 1)
        emit_mult(3, 1)
        emit_ctx(3, Q8F, ctxall)
        emit_mult(4, 1)
        emit_ctx(8, Q8F, ctxall)
        emit_ctx(9, Q8F, ctxall)
        emit_wexp(7, 1, split=((0, 512), (512, 512)))
        emit_mult(5, 1)
        emit_ctx(4, Q8F, ctxall)
        emit_ctx(5, Q8F, ctxall)
        emit_ctx(10, Q8F, ctxall)
        emit_ctx(11, Q8F, ctxall)
        emit_mult(6, 1)
        emit_ctx(6, Q8F, ctxall)
        emit_ctx(7, Q8F, ctxall)
        emit_ctx(12, Q8F, ctxall)
        emit_ctx(13, Q8F, ctxall)
        emit_mult(7, 1)
        emit_ctx(14, Q8F, ctxall)

        # epilogue in two halves so output DMA overlaps the last ctx matmuls
        recip_t = outp.tile([128, SC, 1], F32, tag="recip")
        o_all = outp.tile([128, SC, H], BF16, tag="o")

        def emit_epilogue(qlo, qhi):
            nc.vector.reciprocal(
                recip_t[:, qlo:qhi], ctxall[:, qlo:qhi, H:H + 1]
            )
            rb = bass.AP(
                tensor=recip_t.tensor,
                offset=recip_t.offset + qlo * recip_t.ap[1][0],
                ap=[recip_t.ap[0], [recip_t.ap[1][0], qhi - qlo], [0, H]],
            )
            nc.vector.tensor_tensor(
                o_all[:, qlo:qhi], ctxall[:, qlo:qhi, 0:H], rb, ALU.mult
            )
            nc.sync.dma_start(
                out=dout.ap()[qlo * 128:qhi * 128].rearrange(
                    "(qj p) h -> p qj h", p=128
                ),
                in_=o_all[:, qlo:qhi],
            )

        emit_epilogue(0, 8)
        emit_ctx(15, Q8F, ctxall)
        emit_epilogue(8, SC)

    nc.compile()
    return nc


def get_program():
    if "nc" not in _cache:
        _cache["nc"] = _build_program()
    return _cache["nc"]


def _pack_cols(mat_t):
    """[E, S] -> [128, EC*S] chunk-major so each (partition, chunk) run is
    contiguous in DRAM."""
    out = np.empty((128, EC * S), dtype=mat_t.dtype)
    off = 0
    for lo, ln in CHUNKS:
        blk = mat_t[:, lo:lo + ln].reshape(EC, 128, ln).transpose(1, 0, 2)
        out[:, off:off + EC * ln] = blk.reshape(128, EC * ln)
        off += EC * ln
    return out


def make_in_maps(I, x, mask, Wq, bq, Wk, bk, Wv, bv):
    import ml_dtypes

    BF = ml_dtypes.bfloat16
    I = np.asarray(I, dtype=np.float32)
    x = np.asarray(x, dtype=np.float32)
    mask = np.asarray(mask, dtype=np.int32)

    Wqk = np.concatenate(
        [np.asarray(Wq, np.float32), np.asarray(Wk, np.float32)], axis=1
    ).astype(BF)
    Wv_ = np.asarray(Wv, np.float32).astype(BF)
    bq_ = np.asarray(bq, np.float32)
    bk_ = np.asarray(bk, np.float32)
    bqk = np.concatenate([bq_, bk_]).reshape(128, 1).astype(np.float32)
    bkq = np.concatenate([bk_, bq_]).reshape(128, 1).astype(np.float32)
    bv512 = np.tile(np.asarray(bv, np.float32).reshape(1, H), (1, 8)).astype(BF)

    maps = []
    for b in range(B):
        mt = np.ascontiguousarray(mask[b].T).astype(np.uint8)
        maps.append({
            "ITp": _pack_cols(np.ascontiguousarray(I[b].T).astype(BF)),
            "XTp": _pack_cols(np.ascontiguousarray(x[b].T).astype(BF)),
            "maskTp": np.ascontiguousarray(
                mt.reshape(SC, 128, S).transpose(1, 0, 2).reshape(128, SC * S)
            ),
            "Wqk": Wqk, "Wv": Wv_, "bqk": bqk, "bkq": bkq, "bv512": bv512,
        })
    return maps


def kernel(I, x, mask, Wq, bq, Wk, bk, Wv, bv):
    nc = get_program()
    in_maps = make_in_maps(I, x, mask, Wq, bq, Wk, bk, Wv, bv)
    res = run_bass_kernel_spmd(nc, in_maps, list(range(N_CORES)))
    out = np.stack([res.results[b]["out"] for b in range(B)], axis=0)
    return out.astype(np.float32)


# revision 19
# speedup vs baseline: 1.4854x; 1.1317x over previous
"""Trainium2 Bass kernel for a single DeBERTa-style attention head.

Problem shapes (hardcoded):
  B=8, S=2048, E=768(n_embed), H=64(head)
  q = I @ Wq + bq ; k = x @ Wk + bk ; v = x @ Wv + bv
  w = (q @ k^T) / sqrt(E) ; w = where(mask==0, -1e9, w)
  scores = softmax(w, axis=-1) ; out = scores @ v

Sharding: data-parallel over batch B across the 8 NeuronCores.

Design notes (v1 was ~97.6us):
  * HBM stream per core: I/x host-cast to bf16 (6MB), mask uint8 (4MB,
    cast-DMA to bf16).  All bulk DRAM buffers are HOST-PACKED so each
    (partition, chunk) is one contiguous >=3KB run -> SWDGE descriptors
    stay large and the stream runs near the HBM roofline.  fp8 anywhere
    was measured (numpy) at >=2e-2 rel err -> rejected.
  * exp on ACT is the hard floor (~33us for 4.2M logits at 1/lane/cycle);
    the schedule keeps ACT fed from ~13us: 256-col starter chunks, then
    column-progressive I/x interleave, x^T back half before I^T back half
    (k-side projections unlock score pairs 4-7 half-0), masks last (their
    multiplies are exp-gated anyway).
  * q/k projections col-tiled: lhsT=[Wq|Wk] -> [qT;kT] in one PSUM bank,
    two concurrent 64-col PE tiles; for the front half a second swapped
    pass ([Wk|Wq] -> [kT;qT]) builds the duplicate directly on the PE
    (no cross-partition DMA latency on the critical path); the back half
    duplicates via on-chip SBUF->SBUF DMA (latency hidden by then).
  * score matmuls run 2x row-tiled (K=64): ki_a from partitions 0:64 of
    DUP/QK, ki_b from 64:128, emitted interleaved so the PE overlaps them.
  * exp writes straight into the sT tile; mask multiply is an in-place DVE
    tensor_tensor (2x mode) emitted in mask-arrival order.
  * v projection packs 8 seq-chunks per PSUM bank (2 banks), one rank-1
    bias matmul + one DVE copy per bank; denominators fall out of the 65th
    (ones) column of the ctx matmul.
  * ctx accumulation split per q-half; epilogue in two halves so the output
    DMA overlaps the last ctx matmuls; output bf16, host upcasts.
"""

import math
from contextlib import ExitStack

import numpy as np

import concourse.bass as bass
import concourse.tile as tile
import concourse.mybir as mybir
from concourse import bacc
from concourse.bass_utils import run_bass_kernel_spmd

B, S, E, H = 8, 2048, 768, 64
N_CORES = 8
SC = S // 128   # 16 seq chunks
EC = E // 128   # 6 embed chunks
SCALE = 1.0 / math.sqrt(E)

F32 = mybir.dt.float32
BF16 = mybir.dt.bfloat16
U8 = mybir.dt.uint8
AF = mybir.ActivationFunctionType
ALU = mybir.AluOpType

# column chunks for the I/x streams: (lo, len)
CHUNKS = ((0, 256), (256, 256), (512, 512), (1024, 512), (1536, 512))

_cache = {}


def _build_program():
    nc = bacc.Bacc("TRN2", target_bir_lowering=False, debug=False)

    # host-packed: [128, EC*S] where [p, chunk-major (ec, s)] holds
    # I.T[ec*128+p, lo+s] -- one contiguous run per (partition, chunk)
    dIT = nc.dram_tensor("ITp", [128, EC * S], BF16, kind="ExternalInput")
    dXT = nc.dram_tensor("XTp", [128, EC * S], BF16, kind="ExternalInput")
    # host-packed: [128, SC*S] with [p, ki*S + q] = mask.T[ki*128+p, q]
    dmT = nc.dram_tensor("maskTp", [128, SC * S], U8, kind="ExternalInput")
    dWqk = nc.dram_tensor("Wqk", [E, 128], BF16, kind="ExternalInput")
    dWv = nc.dram_tensor("Wv", [E, H], BF16, kind="ExternalInput")
    dbqk = nc.dram_tensor("bqk", [128, 1], F32, kind="ExternalInput")
    dbkq = nc.dram_tensor("bkq", [128, 1], F32, kind="ExternalInput")
    dbv = nc.dram_tensor("bv512", [1, 512], BF16, kind="ExternalInput")
    dout = nc.dram_tensor("out", [S, H], BF16, kind="ExternalOutput")

    chunk_off = {}
    off = 0
    for lo, ln in CHUNKS:
        chunk_off[lo] = off
        off += EC * ln

    with tile.TileContext(nc) as tc, ExitStack() as ctx:
        singles = ctx.enter_context(tc.tile_pool(name="singles", bufs=1))

        # chunk-major [128, EC*S] mirroring the DRAM packing: loads are
        # 1:1 contiguous copies (large descriptors on BOTH sides)
        IT = singles.tile([128, EC * S], BF16, tag="IT")
        XT = singles.tile([128, EC * S], BF16, tag="XT")

        def load_cols(dst, src, lo, ln):
            o = chunk_off[lo]
            nc.gpsimd.dma_start(
                out=dst[:, o:o + EC * ln], in_=src.ap()[:, o:o + EC * ln]
            )

        def itx(tile_, ei, lo, ln):
            """View of packed I/x: [128, ln] covering embed-chunk ei,
            columns lo:lo+ln (must lie within one load chunk)."""
            for clo, cln in CHUNKS:
                if clo <= lo < clo + cln:
                    assert lo + ln <= clo + cln, (lo, ln)
                    o = chunk_off[clo] + ei * cln + (lo - clo)
                    return tile_[:, o:o + ln]
            raise AssertionError(lo)

        masks = {}
        for mi in range(4):
            masks[mi] = singles.tile(
                [128, 4, S], BF16, name=f"mask{mi}", tag=f"mask{mi}"
            )

        def load_mask(mi):
            nc.gpsimd.dma_start(
                out=masks[mi],
                in_=dmT.ap()[:, mi * 4 * S:(mi + 1) * 4 * S].rearrange(
                    "p (t q) -> p t q", t=4
                ),
            )

        # SWDGE FIFO order == arrival order: column-progressive starters,
        # x^T back half before I^T back half, masks last
        load_cols(IT, dIT, 0, 256)
        load_cols(XT, dXT, 0, 256)
        load_cols(IT, dIT, 256, 256)
        load_cols(XT, dXT, 256, 256)
        load_cols(IT, dIT, 512, 512)
        load_cols(XT, dXT, 512, 512)
        load_mask(0)
        load_cols(XT, dXT, 1024, 512)
        load_cols(XT, dXT, 1536, 512)
        load_cols(IT, dIT, 1024, 512)
        load_cols(IT, dIT, 1536, 512)
        load_mask(1)
        load_mask(2)
        load_mask(3)

        ones_row = singles.tile([1, 512], BF16, tag="ones")
        nc.vector.memset(ones_row, 1.0)

        wqk_sb = singles.tile([128, EC, 128], BF16, tag="Wqk")
        nc.sync.dma_start(
            out=wqk_sb, in_=dWqk.ap().rearrange("(ec p) h -> p ec h", p=128)
        )
        wv_sb = singles.tile([128, EC, H], BF16, tag="Wv")
        nc.sync.dma_start(
            out=wv_sb, in_=dWv.ap().rearrange("(ec p) h -> p ec h", p=128)
        )
        bqk_sb = singles.tile([128, 1], F32, tag="bqk")
        nc.sync.dma_start(out=bqk_sb, in_=dbqk.ap())
        bkq_sb = singles.tile([128, 1], F32, tag="bkq")
        nc.sync.dma_start(out=bkq_sb, in_=dbkq.ap())
        bv_sb = singles.tile([1, 512], BF16, tag="bv512")
        nc.sync.dma_start(out=bv_sb, in_=dbv.ap())

        # QK rows 0:64 = qT, rows 64:128 = kT.  DUP is the partition-swapped
        # copy (rows 0:64 = kT, 64:128 = qT) so both score row-tiles find
        # their operands at the right base partition.
        QK = singles.tile([128, S], BF16, tag="QK")
        DUP = singles.tile([128, S], BF16, tag="DUP")
        vA = singles.tile([128, SC, 66], BF16, tag="vA")
        nc.vector.memset(vA[:, :, H:H + 1], 1.0)

        sp = ctx.enter_context(tc.tile_pool(name="sp", bufs=16))
        # single-bank w tiles, 4 in flight: slot k of score-group g is freed
        # by its exp well before group g+1 reaches the same slot, so the PE
        # w-matmuls hide behind the ACT stream instead of serializing with it
        psw = ctx.enter_context(tc.tile_pool(name="psw", bufs=4, space="PSUM"))
        outp = ctx.enter_context(tc.tile_pool(name="outp", bufs=1))

        def emit_proj(ps2, lo, ln=512, swap=False):
            """Col-tiled projection for columns lo:lo+ln.

            swap=False: [Wq|Wk] -> [qT;kT] into QK.
            swap=True:  [Wk|Wq] -> [kT;qT] into DUP (PE-side duplicate for
            the front half -- no cross-partition DMA on the critical path).
            """
            dst = DUP if swap else QK
            bias = bkq_sb if swap else bqk_sb
            ps = ps2.tile([128, 512], F32, tag="pqk")
            for ei in range(EC):
                # two col-tile accumulation groups on one bank; each clears
                # its own partition range at ei=0
                w_lo = wqk_sb[:, ei, 64:128] if swap else wqk_sb[:, ei, 0:64]
                w_hi = wqk_sb[:, ei, 0:64] if swap else wqk_sb[:, ei, 64:128]
                r_lo = XT if swap else IT
                r_hi = IT if swap else XT
                nc.tensor.matmul(
                    ps[0:64, 0:ln],
                    lhsT=w_lo,
                    rhs=itx(r_lo, ei, lo, ln),
                    start=(ei == 0),
                    stop=(ei == EC - 1),
                    skip_group_check=True,
                )
                nc.tensor.matmul(
                    ps[64:128, 0:ln],
                    lhsT=w_hi,
                    rhs=itx(r_hi, ei, lo, ln),
                    start=(ei == 0),
                    stop=(ei == EC - 1),
                    skip_group_check=True,
                )
            nc.vector.tensor_scalar(
                dst[:, lo:lo + ln], ps[:, 0:ln], bias, None, ALU.add
            )

        def emit_dupproj(half, lo):
            """Back-column projection for one side (half=0: q from I^T,
            half=1: k from x^T), computed TWICE via two col-tiles with the
            same weights so both partition halves (QK and DUP layouts) come
            straight off the PE -- no cross-partition DMA latency."""
            rlo = 64 * half
            ps = psv_pool[0].tile([128, 512], F32, tag="pv")
            src = IT if half == 0 else XT
            for ei in range(EC):
                w = wqk_sb[:, ei, rlo:rlo + 64]
                r = itx(src, ei, lo, 512)
                nc.tensor.matmul(
                    ps[0:64, :], lhsT=w, rhs=r,
                    start=(ei == 0), stop=(ei == EC - 1),
                    skip_group_check=True,
                )
                nc.tensor.matmul(
                    ps[64:128, :], lhsT=w, rhs=r,
                    start=(ei == 0), stop=(ei == EC - 1),
                    skip_group_check=True,
                )
            bias = bqk_sb[rlo:rlo + 64]
            if half == 0:   # qT -> QK rows 0:64, DUP rows 64:128
                nc.vector.tensor_scalar(
                    QK[0:64, lo:lo + 512], ps[0:64, :], bias, None, ALU.add
                )
                nc.vector.tensor_scalar(
                    DUP[64:128, lo:lo + 512], ps[64:128, :], bias, None, ALU.add
                )
            else:           # kT -> DUP rows 0:64, QK rows 64:128
                nc.vector.tensor_scalar(
                    DUP[0:64, lo:lo + 512], ps[0:64, :], bias, None, ALU.add
                )
                nc.vector.tensor_scalar(
                    QK[64:128, lo:lo + 512], ps[64:128, :], bias, None, ALU.add
                )

        def emit_v_bank(psv, vb):
            """v projection for seq chunks 8*vb..8*vb+7 packed in one bank."""
            ps = psv.tile([128, 512], F32, tag="pv")
            for j in range(8):
                kb = vb * 8 + j
                for ei in range(EC):
                    nc.tensor.matmul(
                        ps[:, j * 64:(j + 1) * 64],
                        lhsT=itx(XT, ei, kb * 128, 128),
                        rhs=wv_sb[:, ei, :],
                        start=(j == 0 and ei == 0),
                        stop=False,
                    )
            # one rank-1 bias matmul covers all 8 chunks (bv tiled 8x)
            nc.tensor.matmul(
                ps, lhsT=ones_row[:, 0:128], rhs=bv_sb, start=False, stop=True
            )
            nc.vector.tensor_copy(vA[:, vb * 8:(vb + 1) * 8, 0:H], ps)

        sTs = {}

        def emit_wexp(t, hh, split=False):
            """Row-tiled scores + exp for ki pair (2t, 2t+1), q-half hh.

            Each column sub-chunk gets its own single-bank PSUM tile per ki;
            the two K=64 row-tiles (ki_a at partitions 0:64 of DUP/QK, ki_b
            at 64:128) are emitted back-to-back so the PE overlaps them.
            exp writes straight into the sT tiles; the mask multiply is
            emitted separately in mask-arrival order."""
            ki_a, ki_b = 2 * t, 2 * t + 1
            qlo = hh * 1024
            if ki_a not in sTs:
                sTs[ki_a] = sp.tile([128, S], BF16, name=f"sT{ki_a}", tag="sT")
                sTs[ki_b] = sp.tile([128, S], BF16, name=f"sT{ki_b}", tag="sT")
            subs = split if split else ((0, 512), (512, 512))
            for off, ln in subs:
                wps = {}
                for ki, rlo in ((ki_a, 0), (ki_b, 64)):
                    wps[ki] = psw.tile(
                        [128, 512], F32, name=f"w{ki}_{hh}_{off}", tag="w"
                    )
                    src = DUP if rlo == 0 else QK
                    mov = QK if rlo == 0 else DUP
                    nc.tensor.matmul(
                        wps[ki][:, 0:ln],
                        lhsT=src[rlo:rlo + 64, ki * 128:(ki + 1) * 128],
                        rhs=mov[rlo:rlo + 64, qlo + off:qlo + off + ln],
                        start=True,
                        stop=True,
                    )
                for ki in (ki_a, ki_b):
                    nc.scalar.activation(
                        sTs[ki][:, qlo + off:qlo + off + ln],
                        wps[ki][:, 0:ln],
                        AF.Exp,
                        scale=SCALE,
                    )

        def emit_mult(t, hh):
            """In-place mask multiply for ki pair (2t, 2t+1), q-half hh."""
            qlo = hh * 1024
            for ki in (2 * t, 2 * t + 1):
                nc.vector.tensor_tensor(
                    sTs[ki][:, qlo:qlo + 1024],
                    sTs[ki][:, qlo:qlo + 1024],
                    masks[ki // 4][:, ki % 4, qlo:qlo + 1024],
                    ALU.mult,
                )

        def emit_ctx(ki, qjs, ctxall):
            sT_sb = sTs[ki]
            for qj in qjs:
                nc.tensor.matmul(
                    ctxall[:, qj, 0:H + 1],
                    lhsT=sT_sb[:, qj * 128:(qj + 1) * 128],
                    rhs=vA[:, ki, 0:H + 1],
                    start=(ki == 0 and qj % 4 == 0),
                    stop=(ki == SC - 1 and qj % 4 == 3),
                )

        psv_pool = [None]
        with tc.tile_pool(name="ps2", bufs=2, space="PSUM") as ps2, \
             tc.tile_pool(name="psv", bufs=1, space="PSUM") as psv:
            psv_pool[0] = psv
            # PE warmup: ~3.5us of rank-1 streams during the initial DMA wait
            # flips the HAM clock-gate to 2.4GHz before real work arrives
            for _ in range(8):
                wt = ps2.tile([128, 512], F32, tag="pqk")
                nc.tensor.matmul(
                    wt, lhsT=ones_row[:, 0:128], rhs=ones_row, start=True, stop=True
                )
            emit_proj(ps2, 0, 256)
            emit_proj(ps2, 0, 256, swap=True)
            emit_proj(ps2, 256, 256)
            emit_proj(ps2, 256, 256, swap=True)
            emit_wexp(0, 0, split=((0, 256), (256, 256)))
            emit_wexp(1, 0, split=((0, 512),))
            emit_proj(ps2, 512)
            emit_proj(ps2, 512, 512, swap=True)
            emit_wexp(0, 0, split=((512, 512),))
            emit_wexp(1, 0, split=((512, 512),))
            emit_wexp(2, 0, split=((0, 512), (512, 512)))
            emit_wexp(3, 0, split=((0, 512), (512, 512)))
            emit_v_bank(psv, 0)
            emit_mult(0, 0)          # mask chunk 0 lands mid-prologue
            emit_mult(1, 0)
            emit_dupproj(1, 1024)   # kT back half from x^T (arrives early)
            emit_dupproj(1, 1536)
            emit_v_bank(psv, 1)
            emit_wexp(4, 0)
            emit_wexp(5, 0)
            emit_dupproj(0, 1024)   # qT back half once I^T lands
            emit_dupproj(0, 1536)
            emit_wexp(6, 0)
            emit_wexp(7, 0)

        # prologue PSUM pools closed -> 4 banks free for ctx accumulation
        psctx = ctx.enter_context(tc.tile_pool(name="psctx", bufs=1, space="PSUM"))
        ctxall = psctx.tile([128, SC, 128], F32, tag="ctxall")
        Q07 = tuple(range(8))
        Q8F = tuple(range(8, SC))

        emit_ctx(0, Q07, ctxall)
        emit_ctx(1, Q07, ctxall)
        emit_ctx(2, Q07, ctxall)
        emit_ctx(3, Q07, ctxall)
        emit_wexp(0, 1)
        emit_wexp(1, 1)
        # DVE stream in mask/exp-arrival order so it never head-of-line blocks
        emit_mult(0, 1)
        emit_ctx(0, Q8F, ctxall)
        emit_wexp(2, 1)
        emit_mult(2, 0)
        emit_mult(3, 0)
        emit_ctx(4, Q07, ctxall)
        emit_ctx(5, Q07, ctxall)
        emit_ctx(6, Q07, ctxall)
        emit_ctx(7, Q07, ctxall)
        emit_wexp(3, 1)
        emit_mult(1, 1)
        emit_ctx(1, Q8F, ctxall)
        emit_wexp(4, 1)
        emit_mult(4, 0)
        emit_ctx(8, Q07, ctxall)
        emit_ctx(9, Q07, ctxall)
        emit_mult(2, 1)
        emit_ctx(2, Q8F, ctxall)
        emit_wexp(5, 1)
        emit_mult(5, 0)
        emit_ctx(10, Q07, ctxall)
        emit_ctx(11, Q07, ctxall)
        emit_mult(6, 0)
        emit_mult(7, 0)
        emit_ctx(12, Q07, ctxall)
        emit_ctx(13, Q07, ctxall)
        emit_ctx(14, Q07, ctxall)
        emit_ctx(15, Q07, ctxall)
        emit_wexp(6, 1)
        emit_mult(3, 1)
        emit_ctx(3, Q8F, ctxall)
        emit_mult(4, 1)
        emit_ctx(8, Q8F, ctxall)
        emit_ctx(9, Q8F, ctxall)
        emit_wexp(7, 1, split=((0, 512), (512, 512)))
        emit_mult(5, 1)
        emit_ctx(4, Q8F, ctxall)
        emit_ctx(5, Q8F, ctxall)
        emit_ctx(10, Q8F, ctxall)
        emit_ctx(11, Q8F, ctxall)
        emit_mult(6, 1)
        emit_ctx(6, Q8F, ctxall)
        emit_ctx(7, Q8F, ctxall)
        emit_ctx(12, Q8F, ctxall)
        emit_ctx(13, Q8F, ctxall)
        emit_mult(7, 1)
        emit_ctx(14, Q8F, ctxall)

        # epilogue in two halves so output DMA overlaps the last ctx matmuls
        recip_t = outp.tile([128, SC, 1], F32, tag="recip")
        o_all = outp.tile([128, SC, H], BF16, tag="o")

        def emit_epilogue(qlo, qhi):
            nc.vector.reciprocal(
                recip_t[:, qlo:qhi], ctxall[:, qlo:qhi, H:H + 1]
            )
            rb = bass.AP(
                tensor=recip_t.tensor,
                offset=recip_t.offset + qlo * recip_t.ap[1][0],
                ap=[recip_t.ap[0], [recip_t.ap[1][0], qhi - qlo], [0, H]],
            )
            nc.vector.tensor_tensor(
                o_all[:, qlo:qhi], ctxall[:, qlo:qhi, 0:H], rb, ALU.mult
            )
            nc.sync.dma_start(
                out=dout.ap()[qlo * 128:qhi * 128].rearrange(
                    "(qj p) h -> p qj h", p=128
                ),
                in_=o_all[:, qlo:qhi],
            )

        emit_epilogue(0, 8)
        emit_ctx(15, Q8F, ctxall)
        emit_epilogue(8, SC)

    nc.compile()
    return nc


def get_program():
    if "nc" not in _cache:
        _cache["nc"] = _build_program()
    return _cache["nc"]


def _pack_cols(mat_t):
    """[E, S] -> [128, EC*S] chunk-major so each (partition, chunk) run is
    contiguous in DRAM."""
    out = np.empty((128, EC * S), dtype=mat_t.dtype)
    off = 0
    for lo, ln in CHUNKS:
        blk = mat_t[:, lo:lo + ln].reshape(EC, 128, ln).transpose(1, 0, 2)
        out[:, off:off + EC * ln] = blk.reshape(128, EC * ln)
        off += EC * ln
    return out


def make_in_maps(I, x, mask, Wq, bq, Wk, bk, Wv, bv):
    import ml_dtypes

    BF = ml_dtypes.bfloat16
    I = np.asarray(I, dtype=np.float32)
    x = np.asarray(x, dtype=np.float32)
    mask = np.asarray(mask, dtype=np.int32)

    Wqk = np.concatenate(
        [np.asarray(Wq, np.float32), np.asarray(Wk, np.float32)], axis=1
    ).astype(BF)
    Wv_ = np.asarray(Wv, np.float32).astype(BF)
    bq_ = np.asarray(bq, np.float32)
    bk_ = np.asarray(bk, np.float32)
    bqk = np.concatenate([bq_, bk_]).reshape(128, 1).astype(np.float32)
    bkq = np.concatenate([bk_, bq_]).reshape(128, 1).astype(np.float32)
    bv512 = np.tile(np.asarray(bv, np.float32).reshape(1, H), (1, 8)).astype(BF)

    maps = []
    for b in range(B):
        mt = np.ascontiguousarray(mask[b].T).astype(np.uint8)
        maps.append({
            "ITp": _pack_cols(np.ascontiguousarray(I[b].T).astype(BF)),
            "XTp": _pack_cols(np.ascontiguousarray(x[b].T).astype(BF)),
            "maskTp": np.ascontiguousarray(
                mt.reshape(SC, 128, S).transpose(1, 0, 2).reshape(128, SC * S)
            ),
            "Wqk": Wqk, "Wv": Wv_, "bqk": bqk, "bkq": bkq, "bv512": bv512,
        })
    return maps


def kernel(I, x, mask, Wq, bq, Wk, bk, Wv, bv):
    nc = get_program()
    in_maps = make_in_maps(I, x, mask, Wq, bq, Wk, bk, Wv, bv)
    res = run_bass_kernel_spmd(nc, in_maps, list(range(N_CORES)))
    out = np.stack([res.results[b]["out"] for b in range(B)], axis=0)
    return out.astype(np.float32)


# revision 20
# speedup vs baseline: 1.4899x; 1.0030x over previous
"""Trainium2 Bass kernel for a single DeBERTa-style attention head.

Problem shapes (hardcoded):
  B=8, S=2048, E=768(n_embed), H=64(head)
  q = I @ Wq + bq ; k = x @ Wk + bk ; v = x @ Wv + bv
  w = (q @ k^T) / sqrt(E) ; w = where(mask==0, -1e9, w)
  scores = softmax(w, axis=-1) ; out = scores @ v

Sharding: data-parallel over batch B across the 8 NeuronCores.

Design notes (v1 was ~97.6us):
  * HBM stream per core: I/x host-cast to bf16 (6MB), mask uint8 (4MB,
    cast-DMA to bf16).  All bulk DRAM buffers are HOST-PACKED so each
    (partition, chunk) is one contiguous >=3KB run -> SWDGE descriptors
    stay large and the stream runs near the HBM roofline.  fp8 anywhere
    was measured (numpy) at >=2e-2 rel err -> rejected.
  * exp on ACT is the hard floor (~33us for 4.2M logits at 1/lane/cycle);
    the schedule keeps ACT fed from ~13us: 256-col starter chunks, then
    column-progressive I/x interleave, x^T back half before I^T back half
    (k-side projections unlock score pairs 4-7 half-0), masks last (their
    multiplies are exp-gated anyway).
  * q/k projections col-tiled: lhsT=[Wq|Wk] -> [qT;kT] in one PSUM bank,
    two concurrent 64-col PE tiles; for the front half a second swapped
    pass ([Wk|Wq] -> [kT;qT]) builds the duplicate directly on the PE
    (no cross-partition DMA latency on the critical path); the back half
    duplicates via on-chip SBUF->SBUF DMA (latency hidden by then).
  * score matmuls run 2x row-tiled (K=64): ki_a from partitions 0:64 of
    DUP/QK, ki_b from 64:128, emitted interleaved so the PE overlaps them.
  * exp writes straight into the sT tile; mask multiply is an in-place DVE
    tensor_tensor (2x mode) emitted in mask-arrival order.
  * v projection packs 8 seq-chunks per PSUM bank (2 banks), one rank-1
    bias matmul + one DVE copy per bank; denominators fall out of the 65th
    (ones) column of the ctx matmul.
  * ctx accumulation split per q-half; epilogue in two halves so the output
    DMA overlaps the last ctx matmuls; output bf16, host upcasts.
"""

import math
from contextlib import ExitStack

import numpy as np

import concourse.bass as bass
import concourse.tile as tile
import concourse.mybir as mybir
from concourse import bacc
from concourse.bass_utils import run_bass_kernel_spmd

B, S, E, H = 8, 2048, 768, 64
N_CORES = 8
SC = S // 128   # 16 seq chunks
EC = E // 128   # 6 embed chunks
SCALE = 1.0 / math.sqrt(E)

F32 = mybir.dt.float32
BF16 = mybir.dt.bfloat16
U8 = mybir.dt.uint8
AF = mybir.ActivationFunctionType
ALU = mybir.AluOpType

# column chunks for the I/x streams: (lo, len)
CHUNKS = ((0, 256), (256, 256), (512, 512), (1024, 512), (1536, 512))

_cache = {}


def _build_program():
    nc = bacc.Bacc("TRN2", target_bir_lowering=False, debug=False)

    # host-packed: [128, EC*S] where [p, chunk-major (ec, s)] holds
    # I.T[ec*128+p, lo+s] -- one contiguous run per (partition, chunk)
    dIT = nc.dram_tensor("ITp", [128, EC * S], BF16, kind="ExternalInput")
    dXT = nc.dram_tensor("XTp", [128, EC * S], BF16, kind="ExternalInput")
    # host-packed: [128, SC*S] with [p, ki*S + q] = mask.T[ki*128+p, q]
    dmT = nc.dram_tensor("maskTp", [128, SC * S], U8, kind="ExternalInput")
    dWqk = nc.dram_tensor("Wqk", [E, 128], BF16, kind="ExternalInput")
    dWv = nc.dram_tensor("Wv", [E, H], BF16, kind="ExternalInput")
    dbqk = nc.dram_tensor("bqk", [128, 1], F32, kind="ExternalInput")
    dbkq = nc.dram_tensor("bkq", [128, 1], F32, kind="ExternalInput")
    dbv = nc.dram_tensor("bv512", [1, 512], BF16, kind="ExternalInput")
    dout = nc.dram_tensor("out", [S, H], BF16, kind="ExternalOutput")

    chunk_off = {}
    off = 0
    for lo, ln in CHUNKS:
        chunk_off[lo] = off
        off += EC * ln

    with tile.TileContext(nc) as tc, ExitStack() as ctx:
        singles = ctx.enter_context(tc.tile_pool(name="singles", bufs=1))

        # chunk-major [128, EC*S] mirroring the DRAM packing: loads are
        # 1:1 contiguous copies (large descriptors on BOTH sides)
        IT = singles.tile([128, EC * S], BF16, tag="IT")
        XT = singles.tile([128, EC * S], BF16, tag="XT")

        def load_cols(dst, src, lo, ln):
            o = chunk_off[lo]
            nc.gpsimd.dma_start(
                out=dst[:, o:o + EC * ln], in_=src.ap()[:, o:o + EC * ln]
            )

        def itx(tile_, ei, lo, ln):
            """View of packed I/x: [128, ln] covering embed-chunk ei,
            columns lo:lo+ln (must lie within one load chunk)."""
            for clo, cln in CHUNKS:
                if clo <= lo < clo + cln:
                    assert lo + ln <= clo + cln, (lo, ln)
                    o = chunk_off[clo] + ei * cln + (lo - clo)
                    return tile_[:, o:o + ln]
            raise AssertionError(lo)

        masks = {}
        for mi in range(4):
            masks[mi] = singles.tile(
                [128, 4, S], BF16, name=f"mask{mi}", tag=f"mask{mi}"
            )

        def load_mask(mi):
            nc.gpsimd.dma_start(
                out=masks[mi],
                in_=dmT.ap()[:, mi * 4 * S:(mi + 1) * 4 * S].rearrange(
                    "p (t q) -> p t q", t=4
                ),
            )

        # SWDGE FIFO order == arrival order: column-progressive starters,
        # x^T back half before I^T back half, masks last
        load_cols(IT, dIT, 0, 256)
        load_cols(XT, dXT, 0, 256)
        load_cols(IT, dIT, 256, 256)
        load_cols(XT, dXT, 256, 256)
        load_cols(IT, dIT, 512, 512)
        load_cols(XT, dXT, 512, 512)
        load_mask(0)
        load_cols(XT, dXT, 1024, 512)
        load_cols(XT, dXT, 1536, 512)
        load_cols(IT, dIT, 1024, 512)
        load_cols(IT, dIT, 1536, 512)
        load_mask(1)
        load_mask(2)
        load_mask(3)

        ones_row = singles.tile([1, 512], BF16, tag="ones")
        nc.vector.memset(ones_row, 1.0)

        wqk_sb = singles.tile([128, EC, 128], BF16, tag="Wqk")
        nc.sync.dma_start(
            out=wqk_sb, in_=dWqk.ap().rearrange("(ec p) h -> p ec h", p=128)
        )
        wv_sb = singles.tile([128, EC, H], BF16, tag="Wv")
        nc.sync.dma_start(
            out=wv_sb, in_=dWv.ap().rearrange("(ec p) h -> p ec h", p=128)
        )
        bqk_sb = singles.tile([128, 1], F32, tag="bqk")
        nc.sync.dma_start(out=bqk_sb, in_=dbqk.ap())
        bkq_sb = singles.tile([128, 1], F32, tag="bkq")
        nc.sync.dma_start(out=bkq_sb, in_=dbkq.ap())
        bv_sb = singles.tile([1, 512], BF16, tag="bv512")
        nc.sync.dma_start(out=bv_sb, in_=dbv.ap())

        # QK rows 0:64 = qT, rows 64:128 = kT.  DUP is the partition-swapped
        # copy (rows 0:64 = kT, 64:128 = qT) so both score row-tiles find
        # their operands at the right base partition.
        QK = singles.tile([128, S], BF16, tag="QK")
        DUP = singles.tile([128, S], BF16, tag="DUP")
        vA = singles.tile([128, SC, 66], BF16, tag="vA")
        nc.vector.memset(vA[:, :, H:H + 1], 1.0)

        sp = ctx.enter_context(tc.tile_pool(name="sp", bufs=16))
        # single-bank w tiles, 4 in flight: slot k of score-group g is freed
        # by its exp well before group g+1 reaches the same slot, so the PE
        # w-matmuls hide behind the ACT stream instead of serializing with it
        psw = ctx.enter_context(tc.tile_pool(name="psw", bufs=4, space="PSUM"))
        outp = ctx.enter_context(tc.tile_pool(name="outp", bufs=1))

        def emit_proj(ps2, lo, ln=512, swap=False):
            """Col-tiled projection for columns lo:lo+ln.

            swap=False: [Wq|Wk] -> [qT;kT] into QK.
            swap=True:  [Wk|Wq] -> [kT;qT] into DUP (PE-side duplicate for
            the front half -- no cross-partition DMA on the critical path).
            """
            dst = DUP if swap else QK
            bias = bkq_sb if swap else bqk_sb
            ps = ps2.tile([128, 512], F32, tag="pqk")
            for ei in range(EC):
                # two col-tile accumulation groups on one bank; each clears
                # its own partition range at ei=0
                w_lo = wqk_sb[:, ei, 64:128] if swap else wqk_sb[:, ei, 0:64]
                w_hi = wqk_sb[:, ei, 0:64] if swap else wqk_sb[:, ei, 64:128]
                r_lo = XT if swap else IT
                r_hi = IT if swap else XT
                nc.tensor.matmul(
                    ps[0:64, 0:ln],
                    lhsT=w_lo,
                    rhs=itx(r_lo, ei, lo, ln),
                    start=(ei == 0),
                    stop=(ei == EC - 1),
                    skip_group_check=True,
                )
                nc.tensor.matmul(
                    ps[64:128, 0:ln],
                    lhsT=w_hi,
                    rhs=itx(r_hi, ei, lo, ln),
                    start=(ei == 0),
                    stop=(ei == EC - 1),
                    skip_group_check=True,
                )
            nc.vector.tensor_scalar(
                dst[:, lo:lo + ln], ps[:, 0:ln], bias, None, ALU.add
            )

        def emit_dupproj(half, lo):
            """Back-column projection for one side (half=0: q from I^T,
            half=1: k from x^T), computed TWICE via two col-tiles with the
            same weights so both partition halves (QK and DUP layouts) come
            straight off the PE -- no cross-partition DMA latency."""
            rlo = 64 * half
            ps = psv_pool[0].tile([128, 512], F32, tag="pv")
            src = IT if half == 0 else XT
            for ei in range(EC):
                w = wqk_sb[:, ei, rlo:rlo + 64]
                r = itx(src, ei, lo, 512)
                nc.tensor.matmul(
                    ps[0:64, :], lhsT=w, rhs=r,
                    start=(ei == 0), stop=(ei == EC - 1),
                    skip_group_check=True,
                )
                nc.tensor.matmul(
                    ps[64:128, :], lhsT=w, rhs=r,
                    start=(ei == 0), stop=(ei == EC - 1),
                    skip_group_check=True,
                )
            bias = bqk_sb[rlo:rlo + 64]
            if half == 0:   # qT -> QK rows 0:64, DUP rows 64:128
                nc.vector.tensor_scalar(
                    QK[0:64, lo:lo + 512], ps[0:64, :], bias, None, ALU.add
                )
                nc.vector.tensor_scalar(
                    DUP[64:128, lo:lo + 512], ps[64:128, :], bias, None, ALU.add
                )
            else:           # kT -> DUP rows 0:64, QK rows 64:128
                nc.vector.tensor_scalar(
                    DUP[0:64, lo:lo + 512], ps[0:64, :], bias, None, ALU.add
                )
                nc.vector.tensor_scalar(
                    QK[64:128, lo:lo + 512], ps[64:128, :], bias, None, ALU.add
                )

        def emit_v_bank(psv, vb):
            """v projection for seq chunks 8*vb..8*vb+7 packed in one bank."""
            ps = psv.tile([128, 512], F32, tag="pv")
            for j in range(8):
                kb = vb * 8 + j
                for ei in range(EC):
                    nc.tensor.matmul(
                        ps[:, j * 64:(j + 1) * 64],
                        lhsT=itx(XT, ei, kb * 128, 128),
                        rhs=wv_sb[:, ei, :],
                        start=(j == 0 and ei == 0),
                        stop=False,
                    )
            # one rank-1 bias matmul covers all 8 chunks (bv tiled 8x)
            nc.tensor.matmul(
                ps, lhsT=ones_row[:, 0:128], rhs=bv_sb, start=False, stop=True
            )
            nc.vector.tensor_copy(vA[:, vb * 8:(vb + 1) * 8, 0:H], ps)

        sTs = {}

        def emit_wexp(t, hh, split=False):
            """Row-tiled scores + exp for ki pair (2t, 2t+1), q-half hh.

            Each column sub-chunk gets its own single-bank PSUM tile per ki;
            the two K=64 row-tiles (ki_a at partitions 0:64 of DUP/QK, ki_b
            at 64:128) are emitted back-to-back so the PE overlaps them.
            exp writes straight into the sT tiles; the mask multiply is
            emitted separately in mask-arrival order."""
            ki_a, ki_b = 2 * t, 2 * t + 1
            qlo = hh * 1024
            if ki_a not in sTs:
                sTs[ki_a] = sp.tile([128, S], BF16, name=f"sT{ki_a}", tag="sT")
                sTs[ki_b] = sp.tile([128, S], BF16, name=f"sT{ki_b}", tag="sT")
            subs = split if split else ((0, 512), (512, 512))
            for off, ln in subs:
                wps = {}
                for ki, rlo in ((ki_a, 0), (ki_b, 64)):
                    wps[ki] = psw.tile(
                        [128, 512], F32, name=f"w{ki}_{hh}_{off}", tag="w"
                    )
                    src = DUP if rlo == 0 else QK
                    mov = QK if rlo == 0 else DUP
                    nc.tensor.matmul(
                        wps[ki][:, 0:ln],
                        lhsT=src[rlo:rlo + 64, ki * 128:(ki + 1) * 128],
                        rhs=mov[rlo:rlo + 64, qlo + off:qlo + off + ln],
                        start=True,
                        stop=True,
                    )
                for ki in (ki_a, ki_b):
                    nc.scalar.activation(
                        sTs[ki][:, qlo + off:qlo + off + ln],
                        wps[ki][:, 0:ln],
                        AF.Exp,
                        scale=SCALE,
                    )

        def emit_mult(t, hh):
            """In-place mask multiply for ki pair (2t, 2t+1), q-half hh."""
            qlo = hh * 1024
            for ki in (2 * t, 2 * t + 1):
                nc.vector.tensor_tensor(
                    sTs[ki][:, qlo:qlo + 1024],
                    sTs[ki][:, qlo:qlo + 1024],
                    masks[ki // 4][:, ki % 4, qlo:qlo + 1024],
                    ALU.mult,
                )

        def emit_ctx(ki, qjs, ctxall):
            sT_sb = sTs[ki]
            for qj in qjs:
                nc.tensor.matmul(
                    ctxall[:, qj, 0:H + 1],
                    lhsT=sT_sb[:, qj * 128:(qj + 1) * 128],
                    rhs=vA[:, ki, 0:H + 1],
                    start=(ki == 0 and qj % 4 == 0),
                    stop=(ki == SC - 1 and qj % 4 == 3),
                )

        psv_pool = [None]
        with tc.tile_pool(name="ps2", bufs=2, space="PSUM") as ps2, \
             tc.tile_pool(name="psv", bufs=1, space="PSUM") as psv:
            psv_pool[0] = psv
            # PE warmup: ~3.5us of rank-1 streams during the initial DMA wait
            # flips the HAM clock-gate to 2.4GHz before real work arrives
            for _ in range(8):
                wt = ps2.tile([128, 512], F32, tag="pqk")
                nc.tensor.matmul(
                    wt, lhsT=ones_row[:, 0:128], rhs=ones_row, start=True, stop=True
                )
            emit_proj(ps2, 0, 256)
            emit_proj(ps2, 0, 256, swap=True)
            emit_proj(ps2, 256, 256)
            emit_proj(ps2, 256, 256, swap=True)
            emit_wexp(0, 0, split=((0, 256), (256, 256)))
            emit_wexp(1, 0, split=((0, 512),))
            emit_proj(ps2, 512)
            emit_proj(ps2, 512, 512, swap=True)
            emit_wexp(0, 0, split=((512, 512),))
            emit_wexp(1, 0, split=((512, 512),))
            emit_wexp(2, 0, split=((0, 512), (512, 512)))
            emit_wexp(3, 0, split=((0, 512), (512, 512)))
            emit_v_bank(psv, 0)
            emit_dupproj(1, 1024)   # kT back half from x^T (arrives early)
            emit_dupproj(1, 1536)
            emit_v_bank(psv, 1)
            emit_wexp(4, 0)
            emit_wexp(5, 0)
            emit_dupproj(0, 1024)   # qT back half once I^T lands
            emit_dupproj(0, 1536)
            emit_mult(0, 0)          # mask chunk 0 landed a while ago; the
            emit_mult(1, 0)          # dup copies above outrank it on DVE
            emit_wexp(6, 0)
            emit_wexp(7, 0)

        # prologue PSUM pools closed -> 4 banks free for ctx accumulation
        psctx = ctx.enter_context(tc.tile_pool(name="psctx", bufs=1, space="PSUM"))
        ctxall = psctx.tile([128, SC, 128], F32, tag="ctxall")
        Q07 = tuple(range(8))
        Q8F = tuple(range(8, SC))

        emit_ctx(0, Q07, ctxall)
        emit_ctx(1, Q07, ctxall)
        emit_ctx(2, Q07, ctxall)
        emit_ctx(3, Q07, ctxall)
        emit_wexp(0, 1)
        emit_wexp(1, 1)
        # DVE stream in mask/exp-arrival order so it never head-of-line blocks
        emit_mult(0, 1)
        emit_ctx(0, Q8F, ctxall)
        emit_wexp(2, 1)
        emit_mult(2, 0)
        emit_mult(3, 0)
        emit_ctx(4, Q07, ctxall)
        emit_ctx(5, Q07, ctxall)
        emit_ctx(6, Q07, ctxall)
        emit_ctx(7, Q07, ctxall)
        emit_wexp(3, 1)
        emit_mult(1, 1)
        emit_ctx(1, Q8F, ctxall)
        emit_wexp(4, 1)
        emit_mult(4, 0)
        emit_ctx(8, Q07, ctxall)
        emit_ctx(9, Q07, ctxall)
        emit_mult(2, 1)
        emit_ctx(2, Q8F, ctxall)
        emit_wexp(5, 1)
        emit_mult(5, 0)
        emit_ctx(10, Q07, ctxall)
        emit_ctx(11, Q07, ctxall)
        emit_mult(6, 0)
        emit_mult(7, 0)
        emit_ctx(12, Q07, ctxall)
        emit_ctx(13, Q07, ctxall)
        emit_ctx(14, Q07, ctxall)
        emit_ctx(15, Q07, ctxall)
        emit_wexp(6, 1)
        emit_mult(3, 1)
        emit_ctx(3, Q8F, ctxall)
        emit_mult(4, 1)
        emit_ctx(8, Q8F, ctxall)
        emit_ctx(9, Q8F, ctxall)
        emit_wexp(7, 1, split=((0, 512), (512, 512)))
        emit_mult(5, 1)
        emit_ctx(4, Q8F, ctxall)
        emit_ctx(5, Q8F, ctxall)
        emit_ctx(10, Q8F, ctxall)
        emit_ctx(11, Q8F, ctxall)
        emit_mult(6, 1)
        emit_ctx(6, Q8F, ctxall)
        emit_ctx(7, Q8F, ctxall)
        emit_ctx(12, Q8F, ctxall)
        emit_ctx(13, Q8F, ctxall)
        emit_mult(7, 1)
        emit_ctx(14, Q8F, ctxall)

        # epilogue in two halves so output DMA overlaps the last ctx matmuls
        recip_t = outp.tile([128, SC, 1], F32, tag="recip")
        o_all = outp.tile([128, SC, H], BF16, tag="o")

        def emit_epilogue(qlo, qhi):
            nc.vector.reciprocal(
                recip_t[:, qlo:qhi], ctxall[:, qlo:qhi, H:H + 1]
            )
            rb = bass.AP(
                tensor=recip_t.tensor,
                offset=recip_t.offset + qlo * recip_t.ap[1][0],
                ap=[recip_t.ap[0], [recip_t.ap[1][0], qhi - qlo], [0, H]],
            )
            nc.vector.tensor_tensor(
                o_all[:, qlo:qhi], ctxall[:, qlo:qhi, 0:H], rb, ALU.mult
            )
            nc.sync.dma_start(
                out=dout.ap()[qlo * 128:qhi * 128].rearrange(
                    "(qj p) h -> p qj h", p=128
                ),
                in_=o_all[:, qlo:qhi],
            )

        emit_epilogue(0, 8)
        emit_ctx(15, Q8F, ctxall)
        emit_epilogue(8, SC)

    nc.compile()
    return nc


def get_program():
    if "nc" not in _cache:
        _cache["nc"] = _build_program()
    return _cache["nc"]


def _pack_cols(mat_t):
    """[E, S] -> [128, EC*S] chunk-major so each (partition, chunk) run is
    contiguous in DRAM."""
    out = np.empty((128, EC * S), dtype=mat_t.dtype)
    off = 0
    for lo, ln in CHUNKS:
        blk = mat_t[:, lo:lo + ln].reshape(EC, 128, ln).transpose(1, 0, 2)
        out[:, off:off + EC * ln] = blk.reshape(128, EC * ln)
        off += EC * ln
    return out


def make_in_maps(I, x, mask, Wq, bq, Wk, bk, Wv, bv):
    import ml_dtypes

    BF = ml_dtypes.bfloat16
    I = np.asarray(I, dtype=np.float32)
    x = np.asarray(x, dtype=np.float32)
    mask = np.asarray(mask, dtype=np.int32)

    Wqk = np.concatenate(
        [np.asarray(Wq, np.float32), np.asarray(Wk, np.float32)], axis=1
    ).astype(BF)
    Wv_ = np.asarray(Wv, np.float32).astype(BF)
    bq_ = np.asarray(bq, np.float32)
    bk_ = np.asarray(bk, np.float32)
    bqk = np.concatenate([bq_, bk_]).reshape(128, 1).astype(np.float32)
    bkq = np.concatenate([bk_, bq_]).reshape(128, 1).astype(np.float32)
    bv512 = np.tile(np.asarray(bv, np.float32).reshape(1, H), (1, 8)).astype(BF)

    maps = []
    for b in range(B):
        mt = np.ascontiguousarray(mask[b].T).astype(np.uint8)
        maps.append({
            "ITp": _pack_cols(np.ascontiguousarray(I[b].T).astype(BF)),
            "XTp": _pack_cols(np.ascontiguousarray(x[b].T).astype(BF)),
            "maskTp": np.ascontiguousarray(
                mt.reshape(SC, 128, S).transpose(1, 0, 2).reshape(128, SC * S)
            ),
            "Wqk": Wqk, "Wv": Wv_, "bqk": bqk, "bkq": bkq, "bv512": bv512,
        })
    return maps


def kernel(I, x, mask, Wq, bq, Wk, bk, Wv, bv):
    nc = get_program()
    in_maps = make_in_maps(I, x, mask, Wq, bq, Wk, bk, Wv, bv)
    res = run_bass_kernel_spmd(nc, in_maps, list(range(N_CORES)))
    out = np.stack([res.results[b]["out"] for b in range(B)], axis=0)
    return out.astype(np.float32)
